# revision 20
# baseline (speedup 1.0000x reference)
"""Trainium2 Bass kernel for nn_DecoderStack (2-layer decoder + FFN).

B=4 T=1024 D=1024 H=16 DK=DV=64 FF=4096, fp32 I/O.

Sharding (8 cores): core c -> batch b=c//2, head-group m=c%2 (8 of 16 heads).
Activations kept transposed on device: [d on partitions, t on free].

v3: fp8(e4m3) DoubleRow for every >=128-contraction matmul outside the
attention inner loops (QKV projections both layers, Wo1, Wo2) -- halves
their PE cycles; the masked-softmax weights are exp'd straight into fp8
with the scale folded into the exp bias. aot is stored fp8 so the
AllGather ships half the bytes and Wo1 needs no re-cast. The attention
softmax denominator for head 1 moves from the ACT accumulator to a DVE
reduce (ACT is the attention-phase bottleneck). The FFN consumes RAW
(unnormalized) z2: layernorm is folded into the matmul as a stacked
rank-2 correction (wsum|b_in rows x -mean|std rows) appended after the
column stats land, and the rstd scale is deferred through the ReLU to
the z3 assembly -- so the first FFN matmul issues the moment
ReduceScatter-A lands.

kernel(**inputs) takes full unsharded inputs, returns (x, out3) like the ref.
"""
import contextlib

import numpy as np

import concourse.bass as bass  # noqa: F401
import concourse.tile as tile
from concourse import bacc, mybir

F32 = mybir.dt.float32
BF16 = mybir.dt.bfloat16
F16 = mybir.dt.float16
F8 = mybir.dt.float8e4
AF = mybir.ActivationFunctionType
ALU = mybir.AluOpType
DR = mybir.MatmulPerfMode.DoubleRow

NCORES = 8
B, T, D, H, DK, DV, FF = 4, 1024, 1024, 16, 64, 64, 4096
NCH = D // 128          # 8 d-chunks of 128
NPAIR = 4               # head-pairs per core (8 heads)
THALF = T // 2
TQ = THALF // 2
NFT = FF // 128         # 32 f-tiles
ISQ = float(1.0 / np.sqrt(np.float32(DK)))
INV_D = float(1.0 / D)
INV_D1 = float(1.0 / (D - 1))
D_OVER_D1 = float(D / (D - 1))
FFN_LAG = 10            # B-half trails A by this many f-tiles
NWIN = FFN_LAG + 6      # w_in tile bufs (cover lag + prefetch distance)

# fp8 scale plan
SEW = 128.0             # exp(w)+bias scale for ewq/ewk (via exp bias ln SEW)
LN_SEW = float(np.log(SEW))
SA = 16.0               # activation quant scale (y, x, z1, aot)
SW = 512.0              # weight quant scale (wv1, wq2/wk2/wv2, wo1, wo2)
IS_PROJ = float(1.0 / (SW * SA))    # proj psum descale
SCK = float(SA * SA)    # extra factor folded into ck for L1 scores

REPLICA_GROUPS = [[0, 1], [2, 3], [4, 5], [6, 7]]


# ================================================================ builder ===
def build_nc(debug=False):
    nc = bacc.Bacc("TRN2", target_bir_lowering=False, debug=False,
                   num_devices=NCORES)

    io = {}

    def din(name, shape, dt):
        io[name] = nc.dram_tensor(name, shape, dt, kind="ExternalInput")

    din("yT", [NCH, 128, T], BF16)
    din("y8T", [NCH, 128, T], F8)
    din("x8T", [NCH, 128, T], F8)
    din("wq1", [NCH, 128, 512], BF16)
    din("wk1", [NCH, 128, 512], BF16)
    din("wv1", [NCH, 128, 512], F8)
    din("wo1", [2 * NPAIR, 128, D], F8)
    din("wq2", [NCH, 128, 512], F8)
    din("wk2", [NCH, 128, 512], F8)
    din("wv2", [NCH, 128, 512], F8)
    din("wo2", [NPAIR, 128, D], F8)
    din("win", [NFT, NCH, 128, 128], BF16)
    din("wout", [NCH, NFT, 128, 128], BF16)
    din("corrw", [2, FF], BF16)
    din("bout", [128, NCH], F32)
    din("mask", [128, 128], BF16)

    out_d = nc.dram_tensor("out3T", [NCH, 128, THALF], F32, kind="ExternalOutput")
    dbg = {}
    if debug:
        for name, shape, dt in (
                ("d_z1", [128, NCH, T], BF16),
                ("d_qt2", [128, NPAIR, T], BF16),
                ("d_hp", [128, NCH, T], BF16),
                ("d_z2", [128, NCH, THALF], BF16)):
            dbg[name] = nc.dram_tensor(name, shape, dt, kind="ExternalOutput")

    with tile.TileContext(nc) as tc:
        _emit(nc, tc, io, out_d, dbg)
    nc.compile()
    return nc


def _dump(nc, dbg, name, t_sb):
    if name in dbg:
        nc.sync.dma_start(dbg[name].ap(), t_sb)


def _emit(nc, tc, io, out_d, dbg):
    ctx = contextlib.ExitStack()
    with ctx:
        # ---------------- outer pools (live whole kernel) ----------------
        const = ctx.enter_context(tc.tile_pool(name="const", bufs=1))
        stat = ctx.enter_context(tc.tile_pool(name="stat", bufs=1))
        scr = ctx.enter_context(tc.tile_pool(name="scr", bufs=1))
        dram = ctx.enter_context(tc.tile_pool(name="dram", bufs=1, space="DRAM"))
        psp = ctx.enter_context(tc.tile_pool(name="psp", bufs=1, space="PSUM"))

        def ps_tile(shape=(128, T)):
            return psp.tile(list(shape), F32, tag="ps", bufs=4, name="ps")

        pools = {}

        # ---------------- constants ----------------
        ones_col = const.tile([128, 1], BF16)
        nc.vector.memset(ones_col[:], 1.0)
        ones_row = const.tile([1, 128], F16)
        nc.vector.memset(ones_row[:], 1.0)
        one1 = const.tile([1, 1], F32)
        nc.vector.memset(one1[:], 1.0)
        mask_sb = const.tile([128, 128], BF16)
        nc.sync.dma_start(mask_sb[:], io["mask"].ap())
        lnsew_col = const.tile([128, 1], F32)
        nc.vector.memset(lnsew_col[:], LN_SEW)
        warm1 = const.tile([1, 1], F32)
        nc.scalar.activation(out=warm1[:], in_=one1[:], func=AF.Sqrt)
        corrw_sb = const.tile([2, FF], BF16)
        bout_sb = const.tile([128, NCH], F32)

        # ======== transposed-space layernorm (over d = partitions) =========
        def row16(ps, tw, name):
            r = stat.tile([1, T], F16, tag="r16", bufs=4, name=name)
            nc.vector.tensor_copy(out=r[:, :tw], in_=ps[:, :tw])
            return r

        def bcast16(row, tw):
            r_ps = ps_tile()
            for nh in range(0, tw, 512):
                w = min(512, tw - nh)
                nc.tensor.matmul(r_ps[:, nh:nh + w], ones_row[:],
                                 row[:, nh:nh + w], start=True, stop=True)
            return r_ps

        def stats_mms(z_sb, tw, off=0):
            """PE column sums of z and z^2 -> (s_ps, ss_ps) [1, tw]."""
            s_ps = ps_tile((1, T))
            for c in range(NCH):
                for nh in range(0, tw, 512):
                    w = min(512, tw - nh)
                    nc.tensor.matmul(
                        s_ps[:, nh:nh + w], ones_col[:],
                        z_sb[:, c, off + nh:off + nh + w],
                        start=(c == 0), stop=(c == NCH - 1))
            ss_ps = ps_tile((1, T))
            for c in range(NCH):
                zsq = scr.tile([128, T], BF16, tag="zsq", bufs=2, name="zsq")
                nc.vector.tensor_mul(out=zsq[:, :tw],
                                     in0=z_sb[:, c, off:off + tw],
                                     in1=z_sb[:, c, off:off + tw])
                for nh in range(0, tw, 512):
                    w = min(512, tw - nh)
                    nc.tensor.matmul(
                        ss_ps[:, nh:nh + w], ones_col[:], zsq[:, nh:nh + w],
                        start=(c == 0), stop=(c == NCH - 1))
            return s_ps, ss_ps

        def norm_finish(s_ps, ss_ps, tw):
            """(s16 row, rstd_bc, nmr_bc); wide chain runs on DVE/ACT."""
            s16 = row16(s_ps, tw, "s16")
            ss16 = row16(ss_ps, tw, "ss16")
            s_bc = bcast16(s16, tw)
            ss_bc = bcast16(ss16, tw)
            mean_bc = scr.tile([128, T], F32, tag="s4", bufs=2, name="mean_bc")
            nc.vector.tensor_scalar(out=mean_bc[:, :tw], in0=s_bc[:, :tw],
                                    scalar1=INV_D, scalar2=None, op0=ALU.mult)
            var = scr.tile([128, T], F32, tag="s4", bufs=2, name="var")
            nc.vector.tensor_scalar(out=var[:, :tw], in0=ss_bc[:, :tw],
                                    scalar1=INV_D1, scalar2=None, op0=ALU.mult)
            m2 = scr.tile([128, T], F32, tag="bc", bufs=1, name="m2")
            nc.vector.tensor_scalar(out=m2[:, :tw], in0=mean_bc[:, :tw],
                                    scalar1=D_OVER_D1, scalar2=None,
                                    op0=ALU.mult)
            nc.vector.tensor_tensor(m2[:, :tw], m2[:, :tw], mean_bc[:, :tw],
                                    ALU.mult)
            nc.vector.tensor_tensor(var[:, :tw], var[:, :tw], m2[:, :tw],
                                    ALU.subtract)
            rstd_bc = scr.tile([128, T], F32, tag="nbc", bufs=2,
                               name="rstd_bc")
            if tw <= 256:
                nc.scalar.activation(out=var[:, :tw], in_=var[:, :tw],
                                     func=AF.Sqrt)
                nc.vector.reciprocal(out=rstd_bc[:, :tw], in_=var[:, :tw])
            else:
                nc.scalar.activation(out=var[:, :tw], in_=var[:, :tw],
                                     func=AF.Ln)
                nc.scalar.activation(out=rstd_bc[:, :tw], in_=var[:, :tw],
                                     func=AF.Exp, scale=-0.5)
            nmr_bc = scr.tile([128, T], F32, tag="nbc", bufs=2, name="nmr_bc")
            nc.vector.tensor_mul(out=nmr_bc[:, :tw], in0=mean_bc[:, :tw],
                                 in1=rstd_bc[:, :tw])
            nc.vector.tensor_scalar(out=nmr_bc[:, :tw], in0=nmr_bc[:, :tw],
                                    scalar1=-1.0, scalar2=None, op0=ALU.mult)
            return s16, rstd_bc, nmr_bc

        def norm_apply(z_sb, tw, rstd_bc, nmr_bc, out_sb, chunk_writer=None,
                       off=0, apply_src=None):
            a_src = z_sb if apply_src is None else apply_src
            for c in range(NCH):
                tmp = scr.tile([128, T], F32, tag="ntmp", bufs=1, name="ntmp")
                nc.vector.tensor_mul(out=tmp[:, :tw],
                                     in0=a_src[:, c, off:off + tw],
                                     in1=rstd_bc[:, :tw])
                if chunk_writer is None:
                    nc.vector.tensor_tensor(out_sb[:, c, off:off + tw],
                                            tmp[:, :tw], nmr_bc[:, :tw],
                                            ALU.add)
                else:
                    oc = scr.tile([128, THALF], F32, tag="oc", bufs=2,
                                  name="oc")
                    nc.vector.tensor_tensor(oc[:, :tw], tmp[:, :tw],
                                            nmr_bc[:, :tw], ALU.add)
                    chunk_writer(c, oc[:, :tw])

        # ================= attention inner block (scores/exp/AV) ===========
        # zp (softmax denominator over the t axis): head 0 rides the ACT
        # accumulator, head 1 is a DVE free-axis reduce of the bf16 e tile
        # -- splitting it keeps ACT (the phase bottleneck) lighter.
        def attn_inner(qt_sb, kt_sb, vv_sb, aot_sb, on_pair=None,
                       step_work=None):
            for p in range(NPAIR):
                av_ps = ps_tile()

                def emit_av(st, e_pair, zp):
                    rp = stat.tile([128, 2], F32, tag="rp", bufs=4, name="rp")
                    nc.vector.reciprocal(out=rp[:], in_=zp[:])
                    vv_sc = scr.tile([128, 2, 64], BF16, tag="vvsc", bufs=3,
                                     name="vv_sc")
                    nc.vector.tensor_tensor(
                        vv_sc[:],
                        vv_sb[:, st, 128 * p:128 * (p + 1)].rearrange(
                            "s (h v) -> s h v", h=2),
                        rp[:, :, None].to_broadcast([128, 2, 64]),
                        ALU.mult)
                    for h in range(2):
                        for nh in range(2):
                            nc.tensor.matmul(
                                av_ps[64 * h:64 * (h + 1),
                                      512 * nh:512 * (nh + 1)],
                                vv_sc[:, h, :],
                                e_pair[h][:, 512 * nh:512 * (nh + 1)],
                                start=(st == 0), stop=(st == NCH - 1),
                                tile_position=(0, 64 * h))

                prev = None  # one-step software pipeline
                for st in range(NCH):
                    zp = stat.tile([128, 2], F32, tag="zp", bufs=4, name="zp")
                    e_pair = []
                    for h in range(2):
                        sc_ps = ps_tile()
                        k0 = 64 * h
                        lhsT = kt_sb[k0:k0 + 64, p, 128 * st:128 * (st + 1)]
                        for nh in range(2):
                            nc.tensor.matmul(
                                sc_ps[:, 512 * nh:512 * (nh + 1)], lhsT,
                                qt_sb[k0:k0 + 64, p, 512 * nh:512 * (nh + 1)],
                                start=True, stop=True, tile_position=(k0, 0))
                        e_st = pools["e"].tile([128, T], BF16, tag="E",
                                               bufs=4, name="e_st")
                        nc.scalar.activation(
                            out=e_st[:], in_=sc_ps[:], func=AF.Exp,
                            scale=ISQ, accum_out=zp[:, h:h + 1])
                        e_pair.append(e_st)
                    if prev is not None:
                        emit_av(*prev)
                        if step_work is not None:
                            step_work(p, st - 1)
                    prev = (st, e_pair, zp)
                emit_av(*prev)
                if step_work is not None:
                    step_work(p, NCH - 1)
                nc.vector.tensor_scalar(out=aot_sb[:, p, :], in0=av_ps[:],
                                        scalar1=SA, scalar2=None,
                                        op0=ALU.mult)
                if on_pair is not None:
                    on_pair(p)

        # ================= projection helpers (fp8 DoubleRow) ==============
        def proj_qk_group(dst, w8, src8, fold, p, descale=None):
            pp = ps_tile()
            for c in range(0, NCH, 2):
                lhsT = w8[:, c:c + 2, 128 * p:128 * (p + 1)]
                for nh in range(2):
                    nc.tensor.matmul(
                        pp[:, 512 * nh:512 * (nh + 1)], lhsT,
                        src8[:, c:c + 2, 512 * nh:512 * (nh + 1)],
                        start=(c == 0), stop=(c == NCH - 2), perf_mode=DR)
            if fold is None:
                if descale is None:
                    nc.vector.tensor_copy(out=dst[:, p, :], in_=pp[:])
                else:
                    nc.vector.tensor_scalar(out=dst[:, p, :], in0=pp[:],
                                            scalar1=descale, scalar2=None,
                                            op0=ALU.mult)
            else:
                nc.vector.tensor_scalar(
                    out=dst[:, p, :], in0=pp[:], scalar1=fold[p][:],
                    scalar2=None, op0=ALU.mult)

        def proj_qk(dst, w8, src8, fold, descale=None):
            for p in range(NPAIR):
                proj_qk_group(dst, w8, src8, fold, p, descale)

        def proj_v_group(dst, w8, src8, st, descale):
            vp = ps_tile((128, 512))
            for c in range(0, NCH, 2):
                nc.tensor.matmul(
                    vp[:], src8[:, c:c + 2, 128 * st:128 * (st + 1)],
                    w8[:, c:c + 2, :], start=(c == 0), stop=(c == NCH - 2),
                    perf_mode=DR)
            nc.vector.tensor_scalar(out=dst[:, st, :], in0=vp[:],
                                    scalar1=descale, scalar2=None,
                                    op0=ALU.mult)

        def load_w(pool, name, tag, dt=F8):
            t = pool.tile([128, NCH, 512], dt, tag=tag, name=name + "_sb")
            nc.sync.dma_start(t[:], io[name].ap().rearrange("c p k -> p c k"))
            return t

        # ============================ start =================================
        with tc.tile_pool(name="actA", bufs=1) as actA:  # noqa: F841
            with tc.tile_pool(name="gio", bufs=1) as gio:
                pools["e"] = gio
                y_sb = gio.tile([128, NCH, T], BF16, tag="y", name="y_sb")
                y8_sb = gio.tile([128, NCH, T], F8, tag="y8", name="y8_sb")
                qt = gio.tile([128, NPAIR, T], BF16, tag="qt", name="qt")
                kt = gio.tile([128, NPAIR, T], BF16, tag="kt", name="kt")
                vv = gio.tile([128, NCH, 512], BF16, tag="vv", name="vv")
                vvB = gio.tile([128, NCH, 512], BF16, tag="vvB", name="vvB")
                aot = gio.tile([128, NPAIR, T], F8, tag="aot", name="aot")

                ag_in = dram.tile([NPAIR, 128, T], F8, tag="ag_in",
                                  name="ag_in")
                ag_out1 = dram.tile([2, 2, 128, T], F8, tag="ag_out1",
                                    name="ag_out1")
                ag_out2 = dram.tile([2, 2, 128, T], F8, tag="ag_out2",
                                    name="ag_out2")
                rs_inA = dram.tile([2, NCH, 128, TQ], BF16, tag="rs_inA",
                                   name="rs_inA")
                rs_inB = dram.tile([2, NCH, 128, TQ], BF16, tag="rs_inB",
                                   name="rs_inB")
                rs_outA = dram.tile([NCH, 128, TQ], BF16, tag="rs_outA",
                                    name="rs_outA")
                rs_outB = dram.tile([NCH, 128, TQ], BF16, tag="rs_outB",
                                    name="rs_outB")

                # ---------------- Layer 1 ----------------
                with tc.tile_pool(name="w1", bufs=1) as w1:
                    # masked weight softmax for Wq1/Wk1: exp straight into
                    # fp8 with the SEW scale folded into the exp bias
                    ewq = w1.tile([128, NCH, 512], F8, tag="ewq", name="ewq")
                    ewk = w1.tile([128, NCH, 512], F8, tag="ewk", name="ewk")
                    for nm, ew in (("wq1", ewq), ("wk1", ewk)):
                        raw = w1.tile([128, NCH, 512], BF16, tag="wraw",
                                      bufs=2, name="wraw")
                        for c in range(NCH):  # per-chunk DMA+exp pipeline
                            nc.sync.dma_start(raw[:, c, :], io[nm].ap()[c])
                            nc.scalar.activation(out=ew[:, c, :],
                                                 in_=raw[:, c, :],
                                                 func=AF.Exp,
                                                 bias=lnsew_col[:])
                        nc.vector.tensor_tensor(
                            ew[:, 0, :].rearrange("p (q k) -> p q k", q=NPAIR),
                            ew[:, 0, :].rearrange("p (q k) -> p q k", q=NPAIR),
                            mask_sb[:, None, :].to_broadcast([128, NPAIR, 128]),
                            ALU.mult)
                    for c in range(NCH):  # per-chunk so projections can start
                        nc.sync.dma_start(y_sb[:, c, :], io["yT"].ap()[c])
                        nc.sync.dma_start(y8_sb[:, c, :], io["y8T"].ap()[c])
                    wv1 = load_w(w1, "wv1", "wv")

                    # ck = 1/(SA^2 * colsum(ewq) * colsum(ewk)) per k-feature
                    sq_ps = ps_tile((1, 512))
                    for c in range(NCH):
                        nc.tensor.matmul(sq_ps[:], ones_col[:], ewq[:, c, :],
                                         start=(c == 0), stop=(c == NCH - 1))
                    sk_ps = ps_tile((1, 512))
                    for c in range(NCH):
                        nc.tensor.matmul(sk_ps[:], ones_col[:], ewk[:, c, :],
                                         start=(c == 0), stop=(c == NCH - 1))
                    # fp32 chain: the colsum products (~1e12 with the fp8
                    # scale factors) overflow fp16
                    sq16 = stat.tile([1, 512], F32, tag="sq16", name="sq16")
                    nc.vector.tensor_scalar(out=sq16[:], in0=sq_ps[:],
                                            scalar1=SCK, scalar2=None,
                                            op0=ALU.mult)
                    ck16 = stat.tile([1, 512], F32, tag="ck16", name="ck16")
                    nc.vector.tensor_mul(out=ck16[:], in0=sq16[:],
                                         in1=sk_ps[:])
                    ckT = []
                    for p in range(NPAIR):
                        ct_ps = ps_tile((128, 1))
                        nc.tensor.matmul(ct_ps[:],
                                         ck16[:, 128 * p:128 * (p + 1)],
                                         one1[:], start=True, stop=True)
                        ct = stat.tile([128, 1], F32, tag=f"ckT{p}",
                                       name=f"ckT{p}")
                        nc.vector.reciprocal(out=ct[:], in_=ct_ps[:])
                        ckT.append(ct)

                    proj_qk(qt, ewq, y8_sb, None)
                    proj_qk(kt, ewk, y8_sb, ckT)
                    for st in range(NCH):
                        proj_v_group(vv, wv1, y8_sb, st, IS_PROJ)

                # w1 closed: attn only needs qt/kt/vv; L2 weights reuse its
                # space and their DMAs stream during attn.
                with tc.tile_pool(name="w2", bufs=1) as w2:
                    actB_ctx = tc.tile_pool(name="actB", bufs=1)
                    actB = actB_ctx.__enter__()
                    x8_sb = actB.tile([128, NCH, T], F8, tag="x8",
                                      name="x8_sb")
                    nc.sync.dma_start(
                        x8_sb[:], io["x8T"].ap().rearrange("c p t -> p c t"))
                    nc.sync.dma_start(corrw_sb[:], io["corrw"].ap())
                    nc.sync.dma_start(bout_sb[:], io["bout"].ap())
                    wq2 = load_w(w2, "wq2", "wq2")
                    wk2 = load_w(w2, "wk2", "wk2")
                    wv2 = load_w(w2, "wv2", "wv2")
                    wo2 = w2.tile([128, NPAIR, D], F8, tag="wo2",
                                  name="wo2")
                    nc.sync.dma_start(
                        wo2[:], io["wo2"].ap().rearrange("q p e -> p q e"))
                    wo1f = w2.tile([128, 2, NPAIR, D], F8, tag="wo1f",
                                   name="wo1f")
                    nc.sync.dma_start(
                        wo1f[:], io["wo1"].ap()
                        .rearrange("(r q) p e -> p r q e", r=2))

                    # chunked AllGather of aot (fp8) + L2 K-proj interleaved
                    # into the attention pair loop
                    def ag_hook(p):
                        if p == 1:
                            nc.sync.dma_start(
                                ag_in[0:2].rearrange("q p t -> p q t"),
                                aot[:, 0:2, :])
                            nc.gpsimd.collective_compute(
                                "AllGather", ALU.bypass,
                                replica_groups=REPLICA_GROUPS,
                                ins=[ag_in[0:2].opt()],
                                outs=[ag_out1.opt()])
                        elif p == 3:
                            nc.sync.dma_start(
                                ag_in[2:4].rearrange("q p t -> p q t"),
                                aot[:, 2:4, :])
                            nc.gpsimd.collective_compute(
                                "AllGather", ALU.bypass,
                                replica_groups=REPLICA_GROUPS,
                                ins=[ag_in[2:4].opt()],
                                outs=[ag_out2.opt()])

                    kt2_pp = {}

                    def kt2_step(p, k):
                        # kt[:, g] is free once attn pair g = p-1 is done;
                        # fp8 DoubleRow: one c-pair matmul per (g, k-pair).
                        # vv2 st-groups run whole in steps k==5/7 -- the
                        # filler keeps the PE dense enough that HAM's MID
                        # window never re-throttles the clock.
                        if k in (5, 7):
                            st = 2 * p + (k == 7)
                            proj_v_group(vvB, wv2, x8_sb, st, IS_PROJ)
                            return
                        if p == 0 or k >= 4:
                            return
                        g = p - 1
                        if k == 0:
                            kt2_pp[g] = ps_tile()
                        pp = kt2_pp[g]
                        c = 2 * k
                        lhsT = wk2[:, c:c + 2, 128 * g:128 * (g + 1)]
                        for nh in range(2):
                            nc.tensor.matmul(
                                pp[:, 512 * nh:512 * (nh + 1)], lhsT,
                                x8_sb[:, c:c + 2, 512 * nh:512 * (nh + 1)],
                                start=(k == 0), stop=(k == 3), perf_mode=DR)
                        if k == 3:
                            nc.vector.tensor_scalar(
                                out=kt[:, g, :], in0=kt2_pp.pop(g)[:],
                                scalar1=IS_PROJ, scalar2=None, op0=ALU.mult)

                    attn_inner(qt, kt, vv, aot, on_pair=ag_hook,
                               step_work=kt2_step)

                    # ---- L2 leftovers run during the AllGather flight ----
                    proj_qk_group(kt, wk2, x8_sb, None, 3, descale=IS_PROJ)


                    # colsum(Wq2): rank-1 norm-correction row for qt
                    c2_ps = ps_tile((1, 512))
                    for c in range(NCH):
                        nc.tensor.matmul(c2_ps[:], ones_col[:],
                                         wq2[:, c, :],
                                         start=(c == 0),
                                         stop=(c == NCH - 1))
                    c2q = stat.tile([1, 512], BF16, tag="c2q", name="c2q")
                    nc.vector.tensor_copy(out=c2q[:], in_=c2_ps[:])

                    actB_ctx.__exit__(None, None, None)
                    w2b_ctx = tc.tile_pool(name="w2b", bufs=1)
                    w2b = w2b_ctx.__enter__()

                    # full Wo1 (fp8 DoubleRow over the r pairs) on gathered
                    # heads; z1 = Wo1(aot_full)/S + y in place into y_sb;
                    # z1 column stats + fp8 z1 casts interleave (lag 1)
                    aot_full = w2b.tile([128, 2, NPAIR, T], F8,
                                        tag="aotf", name="aot_full")
                    for r in range(2):
                        nc.sync.dma_start(
                            aot_full[:, r, 0:2, :],
                            ag_out1[r].rearrange("q p t -> p q t"))
                        nc.sync.dma_start(
                            aot_full[:, r, 2:4, :],
                            ag_out2[r].rearrange("q p t -> p q t"))
                    z1 = y_sb  # raw (pre-norm) residual stream
                    z18 = y8_sb  # fp8 copy for the L2 Q projection
                    s1_ps = ps_tile((1, T))
                    ss1_ps = ps_tile((1, T))
                    zsq1 = {}

                    def z1_stats(e, stop):
                        for nh in range(2):
                            nc.tensor.matmul(
                                s1_ps[:, 512 * nh:512 * (nh + 1)], ones_col[:],
                                z1[:, e, 512 * nh:512 * (nh + 1)],
                                start=(e == 0), stop=stop)
                        zq = zsq1.pop(e)
                        for nh in range(2):
                            nc.tensor.matmul(
                                ss1_ps[:, 512 * nh:512 * (nh + 1)],
                                ones_col[:], zq[:, 512 * nh:512 * (nh + 1)],
                                start=(e == 0), stop=stop)

                    for e in range(NCH):
                        wo_ps = ps_tile()
                        for q in range(NPAIR):
                            lhsT = wo1f[:, :, q, 128 * e:128 * (e + 1)]
                            for nh in range(2):
                                nc.tensor.matmul(
                                    wo_ps[:, 512 * nh:512 * (nh + 1)],
                                    lhsT,
                                    aot_full[:, :, q,
                                             512 * nh:512 * (nh + 1)],
                                    start=(q == 0), stop=(q == NPAIR - 1),
                                    perf_mode=DR)
                        wo_sc = scr.tile([128, T], BF16, tag="wosc", bufs=1,
                                         name="wo_sc")
                        nc.scalar.mul(wo_sc[:], wo_ps[:], IS_PROJ)
                        nc.vector.tensor_tensor(y_sb[:, e, :], wo_sc[:],
                                                y_sb[:, e, :], ALU.add)
                        nc.scalar.mul(z18[:, e, :], z1[:, e, :], SA)
                        zq = scr.tile([128, T], BF16, tag="zsq", bufs=2,
                                      name="zsq1")
                        nc.vector.tensor_mul(out=zq[:], in0=z1[:, e, :],
                                             in1=z1[:, e, :])
                        zsq1[e] = zq
                        if e >= 1:
                            z1_stats(e - 1, stop=False)
                    z1_stats(NCH - 1, stop=True)
                    _dump(nc, dbg, "d_z1", z1[:])

                    # fp16 stat rows; negmean (SA-scaled) for the matmul-
                    # folded rank-1 correction
                    s16_1, rstd1_bc, nmr1_bc = norm_finish(s1_ps, ss1_ps, T)
                    negmean = stat.tile([1, T], BF16, tag="negmean",
                                        name="negmean")
                    nc.vector.tensor_scalar(out=negmean[:], in0=s16_1[:],
                                            scalar1=float(-INV_D * SA),
                                            scalar2=None, op0=ALU.mult)
                    rstd1s = scr.tile([128, T], F32, tag="rstd1s", bufs=1,
                                      name="rstd1s")
                    nc.vector.tensor_scalar(out=rstd1s[:], in0=rstd1_bc[:],
                                            scalar1=IS_PROJ, scalar2=None,
                                            op0=ALU.mult)
                    for p in range(NPAIR):
                        pp = ps_tile()
                        for c in range(0, NCH, 2):
                            lhsT = wq2[:, c:c + 2, 128 * p:128 * (p + 1)]
                            for nh in range(2):
                                nc.tensor.matmul(
                                    pp[:, 512 * nh:512 * (nh + 1)], lhsT,
                                    z18[:, c:c + 2, 512 * nh:512 * (nh + 1)],
                                    start=(c == 0), stop=False, perf_mode=DR)
                        for nh in range(2):
                            nc.tensor.matmul(
                                pp[:, 512 * nh:512 * (nh + 1)],
                                c2q[:, 128 * p:128 * (p + 1)],
                                negmean[:, 512 * nh:512 * (nh + 1)],
                                start=False, stop=True)
                        nc.vector.tensor_mul(out=qt[:, p, :], in0=pp[:],
                                             in1=rstd1s[:])
                    _dump(nc, dbg, "d_qt2", qt[:])

                    # half_pre = 0.5*norm(z1): computed chunk-wise on DVE
                    # inside the attn2 pair loop (PE-independent)
                    half_pre = w2b.tile([128, NCH, T], BF16,
                                        tag="hpre", name="half_pre")

                    def hp_work(p):
                        for c in (2 * p, 2 * p + 1):
                            hp_t = scr.tile([128, T], F32, tag="s4", bufs=2,
                                            name="hp_t")
                            nc.vector.tensor_mul(out=hp_t[:],
                                                 in0=z1[:, c, :],
                                                 in1=rstd1_bc[:])
                            nc.vector.tensor_tensor(hp_t[:], hp_t[:],
                                                    nmr1_bc[:], ALU.add)
                            nc.vector.tensor_scalar(
                                out=half_pre[:, c, :], in0=hp_t[:],
                                scalar1=0.5, scalar2=None, op0=ALU.mult)
                        # Wo2 A-piece contribution of pair p, accumulated
                        # (raw psum scale, bf16) into the dead L1 vv tile --
                        # doubles as attn2 PE filler and lets RS-A fire the
                        # moment the pair loop ends
                        for e in range(NCH):
                            wo_ps = ps_tile((128, 512))
                            for i, c0 in enumerate((0, THALF)):
                                nc.tensor.matmul(
                                    wo_ps[:, 256 * i:256 * (i + 1)],
                                    wo2[:, p, 128 * e:128 * (e + 1)],
                                    aot[:, p, c0:c0 + 256],
                                    start=True, stop=True)
                            if p == 0:
                                nc.vector.tensor_copy(out=vv[:, e, :],
                                                      in_=wo_ps[:])
                            else:
                                nc.vector.tensor_tensor(vv[:, e, :],
                                                        wo_ps[:],
                                                        vv[:, e, :], ALU.add)

                    attn_inner(qt, kt, vvB, aot, on_pair=hp_work)
                    _dump(nc, dbg, "d_hp", half_pre[:])

                    # Wo2 partial (fp8 DoubleRow over head-pair pairs) by
                    # t-pieces; RS ops fire as pieces finish
                    def wo2_piece(cols, w, rs_in):
                        for e in range(NCH):
                            wo_ps = ps_tile((128, 512))
                            for i, c0 in enumerate(cols):
                                for q in range(0, NPAIR, 2):
                                    lhsT = wo2[:, q:q + 2,
                                               128 * e:128 * (e + 1)]
                                    nc.tensor.matmul(
                                        wo_ps[:, w * i:w * (i + 1)],
                                        lhsT,
                                        aot[:, q:q + 2, c0:c0 + w],
                                        start=(q == 0),
                                        stop=(q == NPAIR - 2), perf_mode=DR)
                            wo_sc = scr.tile([128, 512], BF16, tag="sb4",
                                             bufs=2, name="wo_sc2")
                            nc.scalar.mul(wo_sc[:], wo_ps[:], IS_PROJ)
                            res = scr.tile([128, 512], BF16, tag="sb4",
                                           bufs=2, name="res")
                            for i, c0 in enumerate(cols):
                                nc.vector.tensor_tensor(
                                    res[:, w * i:w * (i + 1)],
                                    wo_sc[:, w * i:w * (i + 1)],
                                    half_pre[:, e, c0:c0 + w], ALU.add)
                            nc.sync.dma_start(rs_in[0, e], res[:, 0:w])
                            nc.sync.dma_start(rs_in[1, e], res[:, w:2 * w])

                    for e in range(NCH):
                        tmpA = scr.tile([128, 512], BF16, tag="sb4",
                                        bufs=2, name="tmpA")
                        nc.vector.tensor_scalar(out=tmpA[:], in0=vv[:, e, :],
                                                scalar1=IS_PROJ, scalar2=None,
                                                op0=ALU.mult)
                        resA = scr.tile([128, 512], BF16, tag="sb4",
                                        bufs=2, name="resA")
                        for i, c0 in enumerate((0, THALF)):
                            nc.vector.tensor_tensor(
                                resA[:, 256 * i:256 * (i + 1)],
                                tmpA[:, 256 * i:256 * (i + 1)],
                                half_pre[:, e, c0:c0 + 256], ALU.add)
                        nc.sync.dma_start(rs_inA[0, e], resA[:, 0:256])
                        nc.sync.dma_start(rs_inA[1, e], resA[:, 256:512])
                    nc.gpsimd.collective_compute(
                        "ReduceScatter", ALU.add,
                        replica_groups=REPLICA_GROUPS,
                        ins=[rs_inA.opt()], outs=[rs_outA.opt()])
                    wo2_piece((TQ, THALF + TQ), 256, rs_inB)
                    nc.gpsimd.collective_compute(
                        "ReduceScatter", ALU.add,
                        replica_groups=REPLICA_GROUPS,
                        ins=[rs_inB.opt()], outs=[rs_outB.opt()])
                    w2b_ctx.__exit__(None, None, None)

            # ---------------- FFN on local t-half (pre-norm folded) --------
            with tc.tile_pool(name="fh", bufs=1) as fh:
                h_sb = fh.tile([128, NFT, THALF], BF16, tag="h", name="h_sb")
                # z2 halves are SEPARATE tiles so the A-half matmuls carry
                # no (false) dependency on the RS-B landing DMA
                z2A = fh.tile([128, NCH, TQ], BF16, tag="hzA", name="z2A")
                z2B = fh.tile([128, NCH, TQ], BF16, tag="hzB", name="z2B")
                nc.sync.dma_start(z2A[:], rs_outA.rearrange("c p t -> p c t"))
                # correction rows: row0 = -mean(z2), row1 = std(z2); the
                # stacked rank-2 matmul (corrw x rows) folds the layernorm
                # into the w_in matmul; rstd is deferred past the ReLU.
                rows = stat.tile([2, THALF], BF16, tag="rows", name="rows")
                rstd2_bc = fh.tile([128, THALF], F32, tag="r2bc",
                                   name="rstd2_bc")
                nmr2_bc = fh.tile([128, THALF], F32, tag="n2bc",
                                  name="nmr2_bc")

                def z2_half_stats(z2X, off):
                    tw = TQ
                    s_ps, ss_ps = stats_mms(z2X, tw)
                    s16 = row16(s_ps, tw, "s2r")
                    ss16 = row16(ss_ps, tw, "ss2r")
                    mean_r = stat.tile([1, THALF], F32, tag="mean_r", bufs=2,
                                       name="mean_r")
                    nc.vector.tensor_scalar(out=mean_r[:, :tw], in0=s16[:, :tw],
                                            scalar1=INV_D, scalar2=None,
                                            op0=ALU.mult)
                    var_r = stat.tile([1, THALF], F32, tag="var_r", bufs=2,
                                      name="var_r")
                    nc.vector.tensor_scalar(out=var_r[:, :tw],
                                            in0=ss16[:, :tw],
                                            scalar1=INV_D1, scalar2=None,
                                            op0=ALU.mult)
                    m2_r = stat.tile([1, THALF], F32, tag="m2_r", bufs=2,
                                     name="m2_r")
                    nc.vector.tensor_scalar(out=m2_r[:, :tw],
                                            in0=mean_r[:, :tw],
                                            scalar1=D_OVER_D1, scalar2=None,
                                            op0=ALU.mult)
                    nc.vector.tensor_tensor(m2_r[:, :tw], m2_r[:, :tw],
                                            mean_r[:, :tw], ALU.mult)
                    nc.vector.tensor_tensor(var_r[:, :tw], var_r[:, :tw],
                                            m2_r[:, :tw], ALU.subtract)
                    std_r = stat.tile([1, THALF], F32, tag="std_r", bufs=2,
                                      name="std_r")
                    nc.scalar.activation(out=std_r[:, :tw], in_=var_r[:, :tw],
                                         func=AF.Sqrt)
                    nc.vector.tensor_scalar(out=rows[0:1, off:off + tw],
                                            in0=mean_r[:, :tw], scalar1=-1.0,
                                            scalar2=None, op0=ALU.mult)
                    # DVE cannot write at partition base 1; stage + DMA
                    stdb = stat.tile([1, THALF], BF16, tag="stdb", bufs=2,
                                     name="stdb")
                    nc.vector.tensor_copy(out=stdb[:, :tw], in_=std_r[:, :tw])
                    nc.sync.dma_start(rows[1:2, off:off + tw], stdb[:, :tw])
                    rstd_f = stat.tile([1, THALF], F32, tag="rstd_f", bufs=2,
                                       name="rstd_f")
                    nc.vector.reciprocal(out=rstd_f[:, :tw], in_=std_r[:, :tw])
                    rstd_r = stat.tile([1, THALF], F16, tag="rstd_r", bufs=2,
                                       name="rstd_r")
                    nc.vector.tensor_copy(out=rstd_r[:, :tw],
                                          in_=rstd_f[:, :tw])
                    nmr_r = stat.tile([1, THALF], F16, tag="nmr_r", bufs=2,
                                      name="nmr_r")
                    nc.vector.tensor_mul(out=nmr_r[:, :tw],
                                         in0=rows[0:1, off:off + tw],
                                         in1=rstd_r[:, :tw])
                    rb_ps = ps_tile((128, THALF))
                    nc.tensor.matmul(rb_ps[:, :tw], ones_row[:],
                                     rstd_r[:, :tw], start=True, stop=True)
                    nc.vector.tensor_copy(out=rstd2_bc[:, off:off + tw],
                                          in_=rb_ps[:, :tw])
                    nb_ps = ps_tile((128, THALF))
                    nc.tensor.matmul(nb_ps[:, :tw], ones_row[:],
                                     nmr_r[:, :tw], start=True, stop=True)
                    nc.vector.tensor_copy(out=nmr2_bc[:, off:off + tw],
                                          in_=nb_ps[:, :tw])

                z2_half_stats(z2A, 0)
                nc.sync.dma_start(z2B[:], rs_outB.rearrange("c p t -> p c t"))

                pre_wout = fh.tile([128, NFT, 128], BF16, tag="wout", bufs=2,
                                   name="wout_t")
                nc.sync.dma_start(pre_wout[:],
                                  io["wout"].ap()[0].rearrange("f p m -> p f m"))

                win_tiles = {}

                def load_win(ft):
                    wt = fh.tile([128, NCH, 128], BF16, tag="win",
                                 bufs=NWIN, name="win_t")
                    nc.sync.dma_start(
                        wt[:], io["win"].ap()[ft].rearrange("c p f -> p c f"))
                    win_tiles[ft] = wt

                for ft in range(6):
                    load_win(ft)

                def ffn_tile(ft, z2X, lo):
                    hp = ps_tile((128, TQ))
                    win_t = win_tiles[ft]
                    for c in range(NCH):
                        nc.tensor.matmul(hp[:], win_t[:, c, :],
                                         z2X[:, c, :],
                                         start=(c == 0), stop=False)
                    nc.tensor.matmul(hp[:],
                                     corrw_sb[:, 128 * ft:128 * (ft + 1)],
                                     rows[:, lo:lo + TQ], start=False,
                                     stop=True)
                    nc.scalar.activation(out=h_sb[:, ft, lo:lo + TQ],
                                         in_=hp[:],
                                         func=AF.Relu)

                # A-quarter tiles, with B lagging by FFN_LAG (hides RS-B)
                for ft in range(NFT):
                    if ft + 6 < NFT:
                        load_win(ft + 6)
                    ffn_tile(ft, z2A, 0)
                    if ft == FFN_LAG - 2:
                        z2_half_stats(z2B, TQ)
                    if ft >= FFN_LAG:
                        ffn_tile(ft - FFN_LAG, z2B, TQ)
                for ft in range(NFT - FFN_LAG, NFT):
                    ffn_tile(ft, z2B, TQ)

                # wout + deferred-rstd z3 assembly; norm3 stats interleaved
                z3h = (z2A, z2B)  # in-place halves after assembly reads
                z3f = fh.tile([128, NCH, THALF], F32, tag="z3f", name="z3f")
                s3_ps = ps_tile((1, T))
                ss3_ps = ps_tile((1, T))
                zsq3 = {}

                def z3_stats(e, stop):
                    for i, zx in enumerate(z3h):
                        nc.tensor.matmul(s3_ps[:, TQ * i:TQ * (i + 1)],
                                         ones_col[:], zx[:, e, :],
                                         start=(e == 0), stop=stop)
                    zq = zsq3.pop(e)
                    nc.tensor.matmul(ss3_ps[:, :THALF], ones_col[:],
                                     zq[:, :THALF], start=(e == 0), stop=stop)

                for e in range(NCH):
                    if e == 0:
                        wout_t = pre_wout
                    else:
                        wout_t = fh.tile([128, NFT, 128], BF16,
                                         tag="wout", bufs=2, name="wout_t")
                        nc.sync.dma_start(
                            wout_t[:],
                            io["wout"].ap()[e].rearrange("f p m -> p f m"))
                    fp = ps_tile((128, THALF))
                    for fc in range(NFT):
                        nc.tensor.matmul(
                            fp[:], wout_t[:, fc, :], h_sb[:, fc, :],
                            start=(fc == 0), stop=(fc == NFT - 1))
                    # z3 = rstd2*(wout@relu + z2) + nmr2 + bout
                    t1 = scr.tile([128, T], F32, tag="s4", bufs=2,
                                  name="fftmp")
                    nc.vector.tensor_tensor(t1[:, 0:TQ], fp[:, 0:TQ],
                                            z2A[:, e, :], ALU.add)
                    nc.vector.tensor_tensor(t1[:, TQ:THALF], fp[:, TQ:THALF],
                                            z2B[:, e, :], ALU.add)
                    nc.vector.tensor_mul(out=z3f[:, e, :], in0=t1[:, :THALF],
                                         in1=rstd2_bc[:])
                    nc.vector.tensor_tensor(z3f[:, e, :], z3f[:, e, :],
                                            nmr2_bc[:], ALU.add)
                    nc.vector.tensor_scalar(out=z3f[:, e, :],
                                            in0=z3f[:, e, :],
                                            scalar1=bout_sb[:, e:e + 1],
                                            scalar2=None, op0=ALU.add)
                    nc.scalar.mul(z2A[:, e, :], z3f[:, e, 0:TQ], 1.0)
                    nc.scalar.mul(z2B[:, e, :], z3f[:, e, TQ:THALF], 1.0)
                    zq = scr.tile([128, T], BF16, tag="zsq", bufs=2,
                                  name="zsq3")
                    nc.vector.tensor_mul(out=zq[:, :THALF], in0=z3f[:, e, :],
                                         in1=z3f[:, e, :])
                    zsq3[e] = zq
                    if e >= 1:
                        z3_stats(e - 1, stop=False)
                z3_stats(NCH - 1, stop=True)
                _, rstd3_bc, nmr3_bc = norm_finish(s3_ps, ss3_ps, THALF)

                def write_out(c, oc):
                    nc.sync.dma_start(out_d.ap()[c][:, 0:TQ], oc[:, 0:TQ])
                    nc.sync.dma_start(out_d.ap()[c][:, TQ:THALF],
                                      oc[:, TQ:THALF])

                norm_apply(z3f, THALF, rstd3_bc, nmr3_bc, None,
                           chunk_writer=write_out, apply_src=z3f)


# ============================================================== host side ===
def _to_bf16(a):
    import ml_dtypes
    return np.asarray(a, np.float32).astype(ml_dtypes.bfloat16)


def _to_f8(a, scale):
    import ml_dtypes
    a = np.clip(np.asarray(a, np.float32) * scale, -240.0, 240.0)
    return a.astype(ml_dtypes.float8_e4m3)


def _prep_inputs(inputs):
    """Per-core in_maps (host does transposes/tiling/dtype casts only)."""
    x = np.asarray(inputs["x"], np.float32)
    y = np.asarray(inputs["y"], np.float32)
    mask = _to_bf16(np.tile(np.tril(np.ones((128, DK), np.float32)), (1, 2)))
    win_t = _to_bf16(np.asarray(inputs["w_in"], np.float32).T
                     .reshape(NCH, 128, NFT, 128).transpose(2, 0, 1, 3))
    wout_t = _to_bf16(np.asarray(inputs["w_out"], np.float32).T
                      .reshape(NFT, 128, NCH, 128).transpose(2, 0, 1, 3))
    corrw = _to_bf16(np.stack([
        np.asarray(inputs["w_in"], np.float32).sum(axis=1),
        np.asarray(inputs["b_in"], np.float32)]))
    bout2 = np.ascontiguousarray(
        np.asarray(inputs["b_out"], np.float32).reshape(NCH, 128).T)

    def packw(w, hs, f8=False):
        w = np.asarray(w, np.float32)[hs].transpose(1, 0, 2) \
            .reshape(D, 512).reshape(NCH, 128, 512)
        return _to_f8(w, SW) if f8 else _to_bf16(w)

    shared = {"win": win_t, "wout": wout_t, "corrw": corrw, "bout": bout2,
              "mask": mask}
    in_maps = []
    for c in range(NCORES):
        b, m = c // 2, c % 2
        hs = slice(8 * m, 8 * (m + 1))
        im = dict(shared)
        im["yT"] = _to_bf16(y[b].T.reshape(NCH, 128, T))
        im["y8T"] = _to_f8(y[b].T.reshape(NCH, 128, T), SA)
        im["x8T"] = _to_f8(x[b].T.reshape(NCH, 128, T), SA)
        im["wq1"] = packw(inputs["Wq1"], hs)
        im["wk1"] = packw(inputs["Wk1"], hs)
        im["wv1"] = packw(inputs["Wv1"], hs, f8=True)
        im["wq2"] = packw(inputs["Wq2"], hs, f8=True)
        im["wk2"] = packw(inputs["Wk2"], hs, f8=True)
        im["wv2"] = packw(inputs["Wv2"], hs, f8=True)
        im["wo1"] = _to_f8(np.asarray(inputs["Wo1"], np.float32)
                           .reshape(2 * NPAIR, 128, D), SW)
        im["wo2"] = _to_f8(np.asarray(inputs["Wo2"], np.float32)
                           [512 * m:512 * (m + 1)].reshape(NPAIR, 128, D), SW)
        in_maps.append(im)
    return in_maps


def _assemble(results):
    out3 = np.empty((B, T, D), np.float32)
    for b in range(B):
        halves = [results[2 * b + m]["out3T"].reshape(D, THALF)
                  for m in range(2)]
        out3[b] = np.concatenate(halves, axis=1).T
    return out3


# ================================================================ runner ===
_CACHE = {}


def _make_runner(nc, n_cores):
    import jax
    from jax.sharding import Mesh, PartitionSpec
    from jax.experimental.shard_map import shard_map
    from concourse.bass2jax import (_bass_exec_p, install_neuronx_cc_hook,
                                    partition_id_tensor)

    install_neuronx_cc_hook()
    partition_name = nc.partition_id_tensor.name if nc.partition_id_tensor else None
    in_names, out_names, out_avals, zero_outs = [], [], [], []
    for alloc in nc.m.functions[0].allocations:
        if not isinstance(alloc, mybir.MemoryLocationSet):
            continue
        name = alloc.memorylocations[0].name
        if alloc.kind == "ExternalInput":
            if name != partition_name:
                in_names.append(name)
        elif alloc.kind == "ExternalOutput":
            shape = tuple(alloc.tensor_shape)
            dtype = mybir.dt.np(alloc.dtype)
            out_names.append(name)
            out_avals.append(jax.core.ShapedArray(shape, dtype))
            zero_outs.append(np.zeros(shape, dtype))
    n_params = len(in_names)
    n_outs = len(out_avals)
    all_in = in_names + out_names + ([partition_name] if partition_name else [])

    def _body(*args):
        operands = list(args)
        if partition_name is not None:
            operands.append(partition_id_tensor())
        return tuple(_bass_exec_p.bind(
            *operands, out_avals=tuple(out_avals), in_names=tuple(all_in),
            out_names=tuple(out_names), lowering_input_output_aliases=(),
            sim_require_finite=True, sim_require_nnan=True, nc=nc))

    devices = jax.devices()[:n_cores]
    mesh = Mesh(np.asarray(devices), ("core",))
    sharded = jax.jit(
        shard_map(_body, mesh=mesh,
                  in_specs=(PartitionSpec("core"),) * (n_params + n_outs),
                  out_specs=(PartitionSpec("core"),) * n_outs,
                  check_rep=False),
        keep_unused=True)

    def run(in_maps):
        concat_in = [
            np.concatenate([np.asarray(in_maps[c][nm]) for c in range(n_cores)],
                           axis=0)
            for nm in in_names
        ]
        concat_zero = [np.concatenate([z] * n_cores, axis=0) for z in zero_outs]
        outs = [np.asarray(o) for o in sharded(*concat_in, *concat_zero)]
        results = []
        for c in range(n_cores):
            r = {}
            for i, nm in enumerate(out_names):
                per = outs[i].shape[0] // n_cores
                r[nm] = outs[i][c * per:(c + 1) * per]
            results.append(r)
        return results

    return run


def _get_built(debug=False):
    key = "dbg" if debug else "main"
    if key not in _CACHE:
        nc = build_nc(debug=debug)
        run = _make_runner(nc, NCORES)
        _CACHE[key] = (nc, run)
    return _CACHE[key]


def kernel(**inputs):
    nc, run = _get_built()
    in_maps = _prep_inputs(inputs)
    results = run(in_maps)
    out3 = _assemble(results)
    return (np.asarray(inputs["x"], np.float32), out3)


# revision 21
# speedup vs baseline: 1.0400x; 1.0400x over previous
"""Trainium2 Bass kernel for nn_DecoderStack (2-layer decoder + FFN).

B=4 T=1024 D=1024 H=16 DK=DV=64 FF=4096, fp32 I/O.

Sharding (8 cores): core c -> batch b=c//2, head-group m=c%2 (8 of 16 heads).
Activations kept transposed on device: [d on partitions, t on free].

v3: fp8(e4m3) DoubleRow for every >=128-contraction matmul outside the
attention inner loops (QKV projections both layers, Wo1, Wo2) -- halves
their PE cycles; the masked-softmax weights are exp'd straight into fp8
with the scale folded into the exp bias. aot is stored fp8 so the
AllGather ships half the bytes and Wo1 needs no re-cast. The attention
softmax denominator for head 1 moves from the ACT accumulator to a DVE
reduce (ACT is the attention-phase bottleneck). The FFN consumes RAW
(unnormalized) z2: layernorm is folded into the matmul as a stacked
rank-2 correction (wsum|b_in rows x -mean|std rows) appended after the
column stats land, and the rstd scale is deferred through the ReLU to
the z3 assembly -- so the first FFN matmul issues the moment
ReduceScatter-A lands.

kernel(**inputs) takes full unsharded inputs, returns (x, out3) like the ref.
"""
import contextlib

import numpy as np

import concourse.bass as bass  # noqa: F401
import concourse.tile as tile
from concourse import bacc, mybir

F32 = mybir.dt.float32
BF16 = mybir.dt.bfloat16
F16 = mybir.dt.float16
F8 = mybir.dt.float8e4
AF = mybir.ActivationFunctionType
ALU = mybir.AluOpType
DR = mybir.MatmulPerfMode.DoubleRow

NCORES = 8
B, T, D, H, DK, DV, FF = 4, 1024, 1024, 16, 64, 64, 4096
NCH = D // 128          # 8 d-chunks of 128
NPAIR = 4               # head-pairs per core (8 heads)
THALF = T // 2
TQ = THALF // 2
NFT = FF // 128         # 32 f-tiles
ISQ = float(1.0 / np.sqrt(np.float32(DK)))
INV_D = float(1.0 / D)
INV_D1 = float(1.0 / (D - 1))
D_OVER_D1 = float(D / (D - 1))
FFN_LAG = 10            # B-half trails A by this many f-tiles
NWIN = FFN_LAG + 6      # w_in tile bufs (cover lag + prefetch distance)

# fp8 scale plan
SEW = 128.0             # exp(w)+bias scale for ewq/ewk (via exp bias ln SEW)
LN_SEW = float(np.log(SEW))
SA = 16.0               # activation quant scale (y, x, z1, aot)
SW = 512.0              # weight quant scale (wv1, wq2/wk2/wv2, wo1, wo2)
IS_PROJ = float(1.0 / (SW * SA))    # proj psum descale
SCK = float(SA * SA)    # extra factor folded into ck for L1 scores

REPLICA_GROUPS = [[0, 1], [2, 3], [4, 5], [6, 7]]


# ================================================================ builder ===
def build_nc(debug=False):
    nc = bacc.Bacc("TRN2", target_bir_lowering=False, debug=False,
                   num_devices=NCORES)

    io = {}

    def din(name, shape, dt):
        io[name] = nc.dram_tensor(name, shape, dt, kind="ExternalInput")

    din("yT", [NCH, 128, T], BF16)
    din("y8T", [NCH, 128, T], F8)
    din("x8T", [NCH, 128, T], F8)
    din("wq1", [NCH, 128, 512], BF16)
    din("wk1", [NCH, 128, 512], BF16)
    din("wv1", [NCH, 128, 512], F8)
    din("wo1", [2 * NPAIR, 128, D], F8)
    din("wq2", [NCH, 128, 512], F8)
    din("wk2", [NCH, 128, 512], F8)
    din("wv2", [NCH, 128, 512], F8)
    din("wo2", [NPAIR, 128, D], F8)
    din("win", [NFT, NCH, 128, 128], BF16)
    din("wout", [NCH, NFT, 128, 128], BF16)
    din("corrw", [2, FF], BF16)
    din("bout", [128, NCH], F32)
    din("mask", [128, 128], BF16)

    out_d = nc.dram_tensor("out3T", [NCH, 128, THALF], F32, kind="ExternalOutput")
    dbg = {}
    if debug:
        for name, shape, dt in (
                ("d_z1", [128, NCH, T], BF16),
                ("d_qt2", [128, NPAIR, T], BF16),
                ("d_hp", [128, NCH, T], BF16),
                ("d_z2", [128, NCH, THALF], BF16)):
            dbg[name] = nc.dram_tensor(name, shape, dt, kind="ExternalOutput")

    with tile.TileContext(nc) as tc:
        _emit(nc, tc, io, out_d, dbg)
    nc.compile()
    return nc


def _dump(nc, dbg, name, t_sb):
    if name in dbg:
        nc.sync.dma_start(dbg[name].ap(), t_sb)


def _emit(nc, tc, io, out_d, dbg):
    ctx = contextlib.ExitStack()
    with ctx:
        # ---------------- outer pools (live whole kernel) ----------------
        const = ctx.enter_context(tc.tile_pool(name="const", bufs=1))
        stat = ctx.enter_context(tc.tile_pool(name="stat", bufs=1))
        scr = ctx.enter_context(tc.tile_pool(name="scr", bufs=1))
        dram = ctx.enter_context(tc.tile_pool(name="dram", bufs=1, space="DRAM"))
        psp = ctx.enter_context(tc.tile_pool(name="psp", bufs=1, space="PSUM"))

        def ps_tile(shape=(128, T)):
            return psp.tile(list(shape), F32, tag="ps", bufs=4, name="ps")

        pools = {}

        # ---------------- constants ----------------
        ones_col = const.tile([128, 1], BF16)
        nc.vector.memset(ones_col[:], 1.0)
        ones_row = const.tile([1, 128], F16)
        nc.vector.memset(ones_row[:], 1.0)
        one1 = const.tile([1, 1], F32)
        nc.vector.memset(one1[:], 1.0)
        mask_sb = const.tile([128, 128], BF16)
        nc.sync.dma_start(mask_sb[:], io["mask"].ap())
        lnsew_col = const.tile([128, 1], F32)
        nc.vector.memset(lnsew_col[:], LN_SEW)
        warm1 = const.tile([1, 1], F32)
        nc.scalar.activation(out=warm1[:], in_=one1[:], func=AF.Sqrt)
        corrw_sb = const.tile([2, FF], BF16)
        bout_sb = const.tile([128, NCH], F32)

        # ======== transposed-space layernorm (over d = partitions) =========
        def row16(ps, tw, name):
            r = stat.tile([1, T], F16, tag="r16", bufs=4, name=name)
            nc.vector.tensor_copy(out=r[:, :tw], in_=ps[:, :tw])
            return r

        def bcast16(row, tw):
            r_ps = ps_tile()
            for nh in range(0, tw, 512):
                w = min(512, tw - nh)
                nc.tensor.matmul(r_ps[:, nh:nh + w], ones_row[:],
                                 row[:, nh:nh + w], start=True, stop=True)
            return r_ps

        def stats_mms(z_sb, tw, off=0):
            """PE column sums of z and z^2 -> (s_ps, ss_ps) [1, tw]."""
            s_ps = ps_tile((1, T))
            for c in range(NCH):
                for nh in range(0, tw, 512):
                    w = min(512, tw - nh)
                    nc.tensor.matmul(
                        s_ps[:, nh:nh + w], ones_col[:],
                        z_sb[:, c, off + nh:off + nh + w],
                        start=(c == 0), stop=(c == NCH - 1))
            ss_ps = ps_tile((1, T))
            for c in range(NCH):
                zsq = scr.tile([128, T], BF16, tag="zsq", bufs=2, name="zsq")
                nc.vector.tensor_mul(out=zsq[:, :tw],
                                     in0=z_sb[:, c, off:off + tw],
                                     in1=z_sb[:, c, off:off + tw])
                for nh in range(0, tw, 512):
                    w = min(512, tw - nh)
                    nc.tensor.matmul(
                        ss_ps[:, nh:nh + w], ones_col[:], zsq[:, nh:nh + w],
                        start=(c == 0), stop=(c == NCH - 1))
            return s_ps, ss_ps

        def norm_finish(s_ps, ss_ps, tw):
            """(s16 row, rstd_bc, nmr_bc); wide chain runs on DVE/ACT."""
            s16 = row16(s_ps, tw, "s16")
            ss16 = row16(ss_ps, tw, "ss16")
            s_bc = bcast16(s16, tw)
            ss_bc = bcast16(ss16, tw)
            mean_bc = scr.tile([128, T], F32, tag="s4", bufs=2, name="mean_bc")
            nc.vector.tensor_scalar(out=mean_bc[:, :tw], in0=s_bc[:, :tw],
                                    scalar1=INV_D, scalar2=None, op0=ALU.mult)
            var = scr.tile([128, T], F32, tag="s4", bufs=2, name="var")
            nc.vector.tensor_scalar(out=var[:, :tw], in0=ss_bc[:, :tw],
                                    scalar1=INV_D1, scalar2=None, op0=ALU.mult)
            m2 = scr.tile([128, T], F32, tag="bc", bufs=1, name="m2")
            nc.vector.tensor_scalar(out=m2[:, :tw], in0=mean_bc[:, :tw],
                                    scalar1=D_OVER_D1, scalar2=None,
                                    op0=ALU.mult)
            nc.vector.tensor_tensor(m2[:, :tw], m2[:, :tw], mean_bc[:, :tw],
                                    ALU.mult)
            nc.vector.tensor_tensor(var[:, :tw], var[:, :tw], m2[:, :tw],
                                    ALU.subtract)
            rstd_bc = scr.tile([128, T], F32, tag="nbc", bufs=2,
                               name="rstd_bc")
            if tw <= 256:
                nc.scalar.activation(out=var[:, :tw], in_=var[:, :tw],
                                     func=AF.Sqrt)
                nc.vector.reciprocal(out=rstd_bc[:, :tw], in_=var[:, :tw])
            else:
                nc.scalar.activation(out=var[:, :tw], in_=var[:, :tw],
                                     func=AF.Ln)
                nc.scalar.activation(out=rstd_bc[:, :tw], in_=var[:, :tw],
                                     func=AF.Exp, scale=-0.5)
            nmr_bc = scr.tile([128, T], F32, tag="nbc", bufs=2, name="nmr_bc")
            nc.vector.tensor_mul(out=nmr_bc[:, :tw], in0=mean_bc[:, :tw],
                                 in1=rstd_bc[:, :tw])
            nc.vector.tensor_scalar(out=nmr_bc[:, :tw], in0=nmr_bc[:, :tw],
                                    scalar1=-1.0, scalar2=None, op0=ALU.mult)
            return s16, rstd_bc, nmr_bc

        def norm_apply(z_sb, tw, rstd_bc, nmr_bc, out_sb, chunk_writer=None,
                       off=0, apply_src=None):
            a_src = z_sb if apply_src is None else apply_src
            for c in range(NCH):
                tmp = scr.tile([128, T], F32, tag="ntmp", bufs=1, name="ntmp")
                nc.vector.tensor_mul(out=tmp[:, :tw],
                                     in0=a_src[:, c, off:off + tw],
                                     in1=rstd_bc[:, :tw])
                if chunk_writer is None:
                    nc.vector.tensor_tensor(out_sb[:, c, off:off + tw],
                                            tmp[:, :tw], nmr_bc[:, :tw],
                                            ALU.add)
                else:
                    oc = scr.tile([128, THALF], F32, tag="oc", bufs=2,
                                  name="oc")
                    nc.vector.tensor_tensor(oc[:, :tw], tmp[:, :tw],
                                            nmr_bc[:, :tw], ALU.add)
                    chunk_writer(c, oc[:, :tw])

        # ================= attention inner block (scores/exp/AV) ===========
        # zp (softmax denominator over the t axis): head 0 rides the ACT
        # accumulator, head 1 is a DVE free-axis reduce of the bf16 e tile
        # -- splitting it keeps ACT (the phase bottleneck) lighter.
        def attn_inner(qt_sb, kt_sb, vv_sb, aot_sb, on_pair=None,
                       step_work=None):
            for p in range(NPAIR):
                av_ps = ps_tile()

                def emit_av(st, e_pair, zp):
                    rp = stat.tile([128, 2], F32, tag="rp", bufs=4, name="rp")
                    nc.vector.reciprocal(out=rp[:], in_=zp[:])
                    vv_sc = scr.tile([128, 2, 64], BF16, tag="vvsc", bufs=3,
                                     name="vv_sc")
                    nc.vector.tensor_tensor(
                        vv_sc[:],
                        vv_sb[:, st, 128 * p:128 * (p + 1)].rearrange(
                            "s (h v) -> s h v", h=2),
                        rp[:, :, None].to_broadcast([128, 2, 64]),
                        ALU.mult)
                    for h in range(2):
                        for nh in range(2):
                            nc.tensor.matmul(
                                av_ps[64 * h:64 * (h + 1),
                                      512 * nh:512 * (nh + 1)],
                                vv_sc[:, h, :],
                                e_pair[h][:, 512 * nh:512 * (nh + 1)],
                                start=(st == 0), stop=(st == NCH - 1),
                                tile_position=(0, 64 * h))

                prev = None  # one-step software pipeline
                for st in range(NCH):
                    zp = stat.tile([128, 2], F32, tag="zp", bufs=4, name="zp")
                    e_pair = []
                    for h in range(2):
                        sc_ps = ps_tile()
                        k0 = 64 * h
                        lhsT = kt_sb[k0:k0 + 64, p, 128 * st:128 * (st + 1)]
                        for nh in range(2):
                            nc.tensor.matmul(
                                sc_ps[:, 512 * nh:512 * (nh + 1)], lhsT,
                                qt_sb[k0:k0 + 64, p, 512 * nh:512 * (nh + 1)],
                                start=True, stop=True, tile_position=(k0, 0))
                        e_st = pools["e"].tile([128, T], BF16, tag="E",
                                               bufs=4, name="e_st")
                        nc.scalar.activation(
                            out=e_st[:], in_=sc_ps[:], func=AF.Exp,
                            scale=ISQ, accum_out=zp[:, h:h + 1])
                        e_pair.append(e_st)
                    if prev is not None:
                        emit_av(*prev)
                        if step_work is not None:
                            step_work(p, st - 1)
                    prev = (st, e_pair, zp)
                emit_av(*prev)
                if step_work is not None:
                    step_work(p, NCH - 1)
                nc.vector.tensor_scalar(out=aot_sb[:, p, :], in0=av_ps[:],
                                        scalar1=SA, scalar2=None,
                                        op0=ALU.mult)
                if on_pair is not None:
                    on_pair(p)

        # ================= projection helpers (fp8 DoubleRow) ==============
        def proj_qk_group(dst, w8, src8, fold, p, descale=None):
            pp = ps_tile()
            for c in range(0, NCH, 2):
                lhsT = w8[:, c:c + 2, 128 * p:128 * (p + 1)]
                for nh in range(2):
                    nc.tensor.matmul(
                        pp[:, 512 * nh:512 * (nh + 1)], lhsT,
                        src8[:, c:c + 2, 512 * nh:512 * (nh + 1)],
                        start=(c == 0), stop=(c == NCH - 2), perf_mode=DR)
            if fold is None:
                if descale is None:
                    nc.vector.tensor_copy(out=dst[:, p, :], in_=pp[:])
                else:
                    nc.vector.tensor_scalar(out=dst[:, p, :], in0=pp[:],
                                            scalar1=descale, scalar2=None,
                                            op0=ALU.mult)
            else:
                nc.vector.tensor_scalar(
                    out=dst[:, p, :], in0=pp[:], scalar1=fold[p][:],
                    scalar2=None, op0=ALU.mult)

        def proj_qk(dst, w8, src8, fold, descale=None):
            for p in range(NPAIR):
                proj_qk_group(dst, w8, src8, fold, p, descale)

        def proj_v_group(dst, w8, src8, st, descale):
            vp = ps_tile((128, 512))
            for c in range(0, NCH, 2):
                nc.tensor.matmul(
                    vp[:], src8[:, c:c + 2, 128 * st:128 * (st + 1)],
                    w8[:, c:c + 2, :], start=(c == 0), stop=(c == NCH - 2),
                    perf_mode=DR)
            nc.vector.tensor_scalar(out=dst[:, st, :], in0=vp[:],
                                    scalar1=descale, scalar2=None,
                                    op0=ALU.mult)

        def load_w(pool, name, tag, dt=F8):
            t = pool.tile([128, NCH, 512], dt, tag=tag, name=name + "_sb")
            nc.sync.dma_start(t[:], io[name].ap().rearrange("c p k -> p c k"))
            return t

        # ============================ start =================================
        with tc.tile_pool(name="actA", bufs=1) as actA:  # noqa: F841
            with tc.tile_pool(name="gio", bufs=1) as gio:
                pools["e"] = gio
                y_sb = gio.tile([128, NCH, T], BF16, tag="y", name="y_sb")
                y8_sb = gio.tile([128, NCH, T], F8, tag="y8", name="y8_sb")
                qt = gio.tile([128, NPAIR, T], BF16, tag="qt", name="qt")
                kt = gio.tile([128, NPAIR, T], BF16, tag="kt", name="kt")
                vv = gio.tile([128, NCH, 512], BF16, tag="vv", name="vv")
                vvB = gio.tile([128, NCH, 512], BF16, tag="vvB", name="vvB")
                aot = gio.tile([128, NPAIR, T], F8, tag="aot", name="aot")

                ag_in = dram.tile([NPAIR, 128, T], F8, tag="ag_in",
                                  name="ag_in")
                ag_out1 = dram.tile([2, 2, 128, T], F8, tag="ag_out1",
                                    name="ag_out1")
                ag_out2 = dram.tile([2, 2, 128, T], F8, tag="ag_out2",
                                    name="ag_out2")
                rs_inA = dram.tile([2, NCH, 128, TQ], BF16, tag="rs_inA",
                                   name="rs_inA")
                rs_inB = dram.tile([2, NCH, 128, TQ], BF16, tag="rs_inB",
                                   name="rs_inB")
                rs_outA = dram.tile([NCH, 128, TQ], BF16, tag="rs_outA",
                                    name="rs_outA")
                rs_outB = dram.tile([NCH, 128, TQ], BF16, tag="rs_outB",
                                    name="rs_outB")

                # ---------------- Layer 1 ----------------
                with tc.tile_pool(name="w1", bufs=1) as w1:
                    # masked weight softmax for Wq1/Wk1: exp straight into
                    # fp8 with the SEW scale folded into the exp bias
                    ewq = w1.tile([128, NCH, 512], F8, tag="ewq", name="ewq")
                    ewk = w1.tile([128, NCH, 512], F8, tag="ewk", name="ewk")
                    for nm, ew in (("wq1", ewq), ("wk1", ewk)):
                        raw = w1.tile([128, NCH, 512], BF16, tag="wraw",
                                      bufs=2, name="wraw")
                        for c in range(NCH):  # per-chunk DMA+exp pipeline
                            nc.sync.dma_start(raw[:, c, :], io[nm].ap()[c])
                            nc.scalar.activation(out=ew[:, c, :],
                                                 in_=raw[:, c, :],
                                                 func=AF.Exp,
                                                 bias=lnsew_col[:])
                        nc.vector.tensor_tensor(
                            ew[:, 0, :].rearrange("p (q k) -> p q k", q=NPAIR),
                            ew[:, 0, :].rearrange("p (q k) -> p q k", q=NPAIR),
                            mask_sb[:, None, :].to_broadcast([128, NPAIR, 128]),
                            ALU.mult)
                    for c in range(NCH):  # per-chunk so projections can start
                        nc.sync.dma_start(y_sb[:, c, :], io["yT"].ap()[c])
                        nc.sync.dma_start(y8_sb[:, c, :], io["y8T"].ap()[c])
                    wv1 = load_w(w1, "wv1", "wv")

                    # ck = 1/(SA^2 * colsum(ewq) * colsum(ewk)) per k-feature
                    sq_ps = ps_tile((1, 512))
                    for c in range(NCH):
                        nc.tensor.matmul(sq_ps[:], ones_col[:], ewq[:, c, :],
                                         start=(c == 0), stop=(c == NCH - 1))
                    sk_ps = ps_tile((1, 512))
                    for c in range(NCH):
                        nc.tensor.matmul(sk_ps[:], ones_col[:], ewk[:, c, :],
                                         start=(c == 0), stop=(c == NCH - 1))
                    # fp32 chain: the colsum products (~1e12 with the fp8
                    # scale factors) overflow fp16
                    sq16 = stat.tile([1, 512], F32, tag="sq16", name="sq16")
                    nc.vector.tensor_scalar(out=sq16[:], in0=sq_ps[:],
                                            scalar1=SCK, scalar2=None,
                                            op0=ALU.mult)
                    ck16 = stat.tile([1, 512], F32, tag="ck16", name="ck16")
                    nc.vector.tensor_mul(out=ck16[:], in0=sq16[:],
                                         in1=sk_ps[:])
                    ckT = []
                    for p in range(NPAIR):
                        ct_ps = ps_tile((128, 1))
                        nc.tensor.matmul(ct_ps[:],
                                         ck16[:, 128 * p:128 * (p + 1)],
                                         one1[:], start=True, stop=True)
                        ct = stat.tile([128, 1], F32, tag=f"ckT{p}",
                                       name=f"ckT{p}")
                        nc.vector.reciprocal(out=ct[:], in_=ct_ps[:])
                        ckT.append(ct)

                    proj_qk(qt, ewq, y8_sb, None)
                    proj_qk(kt, ewk, y8_sb, ckT)
                    for st in range(NCH):
                        proj_v_group(vv, wv1, y8_sb, st, IS_PROJ)

                # w1 closed: attn only needs qt/kt/vv; L2 weights reuse its
                # space and their DMAs stream during attn.
                with tc.tile_pool(name="w2", bufs=1) as w2:
                    actB_ctx = tc.tile_pool(name="actB", bufs=1)
                    actB = actB_ctx.__enter__()
                    x8_sb = actB.tile([128, NCH, T], F8, tag="x8",
                                      name="x8_sb")
                    nc.sync.dma_start(
                        x8_sb[:], io["x8T"].ap().rearrange("c p t -> p c t"))
                    nc.sync.dma_start(corrw_sb[:], io["corrw"].ap())
                    nc.sync.dma_start(bout_sb[:], io["bout"].ap())
                    wq2 = load_w(w2, "wq2", "wq2")
                    wk2 = load_w(w2, "wk2", "wk2")
                    wv2 = load_w(w2, "wv2", "wv2")
                    wo2 = w2.tile([128, NPAIR, D], F8, tag="wo2",
                                  name="wo2")
                    nc.sync.dma_start(
                        wo2[:], io["wo2"].ap().rearrange("q p e -> p q e"))
                    wo1f = w2.tile([128, 2, NPAIR, D], F8, tag="wo1f",
                                   name="wo1f")
                    nc.sync.dma_start(
                        wo1f[:], io["wo1"].ap()
                        .rearrange("(r q) p e -> p r q e", r=2))

                    # chunked AllGather of aot (fp8) + L2 K-proj interleaved
                    # into the attention pair loop
                    def ag_hook(p):
                        if p == 1:
                            nc.sync.dma_start(
                                ag_in[0:2].rearrange("q p t -> p q t"),
                                aot[:, 0:2, :])
                            nc.gpsimd.collective_compute(
                                "AllGather", ALU.bypass,
                                replica_groups=REPLICA_GROUPS,
                                ins=[ag_in[0:2].opt()],
                                outs=[ag_out1.opt()])
                        elif p == 3:
                            nc.sync.dma_start(
                                ag_in[2:4].rearrange("q p t -> p q t"),
                                aot[:, 2:4, :])
                            nc.gpsimd.collective_compute(
                                "AllGather", ALU.bypass,
                                replica_groups=REPLICA_GROUPS,
                                ins=[ag_in[2:4].opt()],
                                outs=[ag_out2.opt()])

                    kt2_pp = {}

                    def kt2_step(p, k):
                        # kt[:, g] is free once attn pair g = p-1 is done;
                        # fp8 DoubleRow: one c-pair matmul per (g, k-pair).
                        # vv2 st-groups run whole in steps k==5/7 -- the
                        # filler keeps the PE dense enough that HAM's MID
                        # window never re-throttles the clock.
                        if k in (5, 7):
                            st = 2 * p + (k == 7)
                            proj_v_group(vvB, wv2, x8_sb, st, IS_PROJ)
                            return
                        if p == 0 or k >= 4:
                            return
                        g = p - 1
                        if k == 0:
                            kt2_pp[g] = ps_tile()
                        pp = kt2_pp[g]
                        c = 2 * k
                        lhsT = wk2[:, c:c + 2, 128 * g:128 * (g + 1)]
                        for nh in range(2):
                            nc.tensor.matmul(
                                pp[:, 512 * nh:512 * (nh + 1)], lhsT,
                                x8_sb[:, c:c + 2, 512 * nh:512 * (nh + 1)],
                                start=(k == 0), stop=(k == 3), perf_mode=DR)
                        if k == 3:
                            nc.vector.tensor_scalar(
                                out=kt[:, g, :], in0=kt2_pp.pop(g)[:],
                                scalar1=IS_PROJ, scalar2=None, op0=ALU.mult)

                    attn_inner(qt, kt, vv, aot, on_pair=ag_hook,
                               step_work=kt2_step)

                    # ---- L2 leftovers run during the AllGather flight ----
                    proj_qk_group(kt, wk2, x8_sb, None, 3, descale=IS_PROJ)


                    # colsum(Wq2): rank-1 norm-correction row for qt
                    c2_ps = ps_tile((1, 512))
                    for c in range(NCH):
                        nc.tensor.matmul(c2_ps[:], ones_col[:],
                                         wq2[:, c, :],
                                         start=(c == 0),
                                         stop=(c == NCH - 1))
                    c2q = stat.tile([1, 512], BF16, tag="c2q", name="c2q")
                    nc.vector.tensor_copy(out=c2q[:], in_=c2_ps[:])

                    actB_ctx.__exit__(None, None, None)
                    w2b_ctx = tc.tile_pool(name="w2b", bufs=1)
                    w2b = w2b_ctx.__enter__()

                    # full Wo1 (fp8 DoubleRow over the r pairs) on gathered
                    # heads; z1 = Wo1(aot_full)/S + y in place into y_sb;
                    # z1 column stats + fp8 z1 casts interleave (lag 1)
                    aot_full = w2b.tile([128, 2, NPAIR, T], F8,
                                        tag="aotf", name="aot_full")
                    for r in range(2):
                        nc.sync.dma_start(
                            aot_full[:, r, 0:2, :],
                            ag_out1[r].rearrange("q p t -> p q t"))
                        nc.sync.dma_start(
                            aot_full[:, r, 2:4, :],
                            ag_out2[r].rearrange("q p t -> p q t"))
                    z1 = y_sb  # raw (pre-norm) residual stream
                    z18 = y8_sb  # fp8 copy for the L2 Q projection
                    s1_ps = ps_tile((1, T))
                    ss1_ps = ps_tile((1, T))
                    zsq1 = {}

                    def z1_stats(e, stop):
                        for nh in range(2):
                            nc.tensor.matmul(
                                s1_ps[:, 512 * nh:512 * (nh + 1)], ones_col[:],
                                z1[:, e, 512 * nh:512 * (nh + 1)],
                                start=(e == 0), stop=stop)
                        zq = zsq1.pop(e)
                        for nh in range(2):
                            nc.tensor.matmul(
                                ss1_ps[:, 512 * nh:512 * (nh + 1)],
                                ones_col[:], zq[:, 512 * nh:512 * (nh + 1)],
                                start=(e == 0), stop=stop)

                    for e in range(NCH):
                        wo_ps = ps_tile()
                        for q in range(NPAIR):
                            lhsT = wo1f[:, :, q, 128 * e:128 * (e + 1)]
                            for nh in range(2):
                                nc.tensor.matmul(
                                    wo_ps[:, 512 * nh:512 * (nh + 1)],
                                    lhsT,
                                    aot_full[:, :, q,
                                             512 * nh:512 * (nh + 1)],
                                    start=(q == 0), stop=(q == NPAIR - 1),
                                    perf_mode=DR)
                        wo_sc = scr.tile([128, T], BF16, tag="wosc", bufs=1,
                                         name="wo_sc")
                        nc.scalar.mul(wo_sc[:], wo_ps[:], IS_PROJ)
                        nc.vector.tensor_tensor(y_sb[:, e, :], wo_sc[:],
                                                y_sb[:, e, :], ALU.add)
                        nc.scalar.mul(z18[:, e, :], z1[:, e, :], SA)
                        zq = scr.tile([128, T], BF16, tag="zsq", bufs=2,
                                      name="zsq1")
                        nc.vector.tensor_mul(out=zq[:], in0=z1[:, e, :],
                                             in1=z1[:, e, :])
                        zsq1[e] = zq
                        if e >= 1:
                            z1_stats(e - 1, stop=False)
                    z1_stats(NCH - 1, stop=True)
                    _dump(nc, dbg, "d_z1", z1[:])

                    # fp16 stat rows; negmean (SA-scaled) for the matmul-
                    # folded rank-1 correction
                    s16_1, rstd1_bc, nmr1_bc = norm_finish(s1_ps, ss1_ps, T)
                    negmean = stat.tile([1, T], BF16, tag="negmean",
                                        name="negmean")
                    nc.vector.tensor_scalar(out=negmean[:], in0=s16_1[:],
                                            scalar1=float(-INV_D * SA),
                                            scalar2=None, op0=ALU.mult)
                    rstd1s = scr.tile([128, T], F32, tag="rstd1s", bufs=1,
                                      name="rstd1s")
                    nc.vector.tensor_scalar(out=rstd1s[:], in0=rstd1_bc[:],
                                            scalar1=IS_PROJ, scalar2=None,
                                            op0=ALU.mult)
                    for p in range(NPAIR):
                        pp = ps_tile()
                        for c in range(0, NCH, 2):
                            lhsT = wq2[:, c:c + 2, 128 * p:128 * (p + 1)]
                            for nh in range(2):
                                nc.tensor.matmul(
                                    pp[:, 512 * nh:512 * (nh + 1)], lhsT,
                                    z18[:, c:c + 2, 512 * nh:512 * (nh + 1)],
                                    start=(c == 0), stop=False, perf_mode=DR)
                        for nh in range(2):
                            nc.tensor.matmul(
                                pp[:, 512 * nh:512 * (nh + 1)],
                                c2q[:, 128 * p:128 * (p + 1)],
                                negmean[:, 512 * nh:512 * (nh + 1)],
                                start=False, stop=True)
                        nc.vector.tensor_mul(out=qt[:, p, :], in0=pp[:],
                                             in1=rstd1s[:])
                    _dump(nc, dbg, "d_qt2", qt[:])

                    # half_pre = 0.5*norm(z1): computed chunk-wise on DVE
                    # inside the attn2 pair loop (PE-independent)
                    half_pre = w2b.tile([128, NCH, T], BF16,
                                        tag="hpre", name="half_pre")

                    def hp_work(p):
                        for c in (2 * p, 2 * p + 1):
                            hp_t = scr.tile([128, T], F32, tag="s4", bufs=2,
                                            name="hp_t")
                            nc.vector.tensor_mul(out=hp_t[:],
                                                 in0=z1[:, c, :],
                                                 in1=rstd1_bc[:])
                            nc.vector.tensor_tensor(hp_t[:], hp_t[:],
                                                    nmr1_bc[:], ALU.add)
                            nc.vector.tensor_scalar(
                                out=half_pre[:, c, :], in0=hp_t[:],
                                scalar1=float(0.5 * SW * SA), scalar2=None,
                                op0=ALU.mult)

                    # Wo2 A-piece contribution of pair p-1, one e-chunk per
                    # step (raw psum scale, bf16-accumulated into the dead
                    # L1 vv tile). Spreading it per-step keeps the DVE FIFO
                    # short ahead of the latency-critical rp/vv_sc ops, and
                    # the PE filler fights the HAM cold state. RS-A then
                    # fires right after the pair loop.
                    def wo2A_step(p, k):
                        if p == 0:
                            return
                        g, e = p - 1, k
                        wo_ps = ps_tile((128, 512))
                        for i, c0 in enumerate((0, THALF)):
                            nc.tensor.matmul(
                                wo_ps[:, 256 * i:256 * (i + 1)],
                                wo2[:, g, 128 * e:128 * (e + 1)],
                                aot[:, g, c0:c0 + 256],
                                start=True, stop=True)
                        if g == 0:
                            nc.vector.tensor_copy(out=vv[:, e, :],
                                                  in_=wo_ps[:])
                        else:
                            nc.vector.tensor_tensor(vv[:, e, :], wo_ps[:],
                                                    vv[:, e, :], ALU.add)

                    attn_inner(qt, kt, vvB, aot, on_pair=hp_work,
                               step_work=wo2A_step)
                    _dump(nc, dbg, "d_hp", half_pre[:])

                    # Wo2 partial (fp8 DoubleRow over head-pair pairs) by
                    # t-pieces; RS ops fire as pieces finish
                    def wo2_piece(cols, w, rs_in):
                        for e in range(NCH):
                            wo_ps = ps_tile((128, 512))
                            for i, c0 in enumerate(cols):
                                for q in range(0, NPAIR, 2):
                                    lhsT = wo2[:, q:q + 2,
                                               128 * e:128 * (e + 1)]
                                    nc.tensor.matmul(
                                        wo_ps[:, w * i:w * (i + 1)],
                                        lhsT,
                                        aot[:, q:q + 2, c0:c0 + w],
                                        start=(q == 0),
                                        stop=(q == NPAIR - 2), perf_mode=DR)
                            res = scr.tile([128, 512], BF16, tag="sb4",
                                           bufs=2, name="res")
                            for i, c0 in enumerate(cols):
                                nc.vector.tensor_tensor(
                                    res[:, w * i:w * (i + 1)],
                                    wo_ps[:, w * i:w * (i + 1)],
                                    half_pre[:, e, c0:c0 + w], ALU.add)
                            nc.sync.dma_start(rs_in[0, e], res[:, 0:w])
                            nc.sync.dma_start(rs_in[1, e], res[:, w:2 * w])

                    for e in range(NCH):
                        wo_ps = ps_tile((128, 512))
                        for i, c0 in enumerate((0, THALF)):
                            nc.tensor.matmul(
                                wo_ps[:, 256 * i:256 * (i + 1)],
                                wo2[:, 3, 128 * e:128 * (e + 1)],
                                aot[:, 3, c0:c0 + 256],
                                start=True, stop=True)
                        nc.vector.tensor_tensor(vv[:, e, :], wo_ps[:],
                                                vv[:, e, :], ALU.add)
                        resA = scr.tile([128, 512], BF16, tag="sb4",
                                        bufs=2, name="resA")
                        for i, c0 in enumerate((0, THALF)):
                            nc.vector.tensor_tensor(
                                resA[:, 256 * i:256 * (i + 1)],
                                vv[:, e, 256 * i:256 * (i + 1)],
                                half_pre[:, e, c0:c0 + 256], ALU.add)
                        nc.sync.dma_start(rs_inA[0, e], resA[:, 0:256])
                        nc.sync.dma_start(rs_inA[1, e], resA[:, 256:512])
                    nc.gpsimd.collective_compute(
                        "ReduceScatter", ALU.add,
                        replica_groups=REPLICA_GROUPS,
                        ins=[rs_inA.opt()], outs=[rs_outA.opt()])
                    wo2_piece((TQ, THALF + TQ), 256, rs_inB)
                    nc.gpsimd.collective_compute(
                        "ReduceScatter", ALU.add,
                        replica_groups=REPLICA_GROUPS,
                        ins=[rs_inB.opt()], outs=[rs_outB.opt()])
                    w2b_ctx.__exit__(None, None, None)

            # ---------------- FFN on local t-half (pre-norm folded) --------
            with tc.tile_pool(name="fh", bufs=1) as fh:
                h_sb = fh.tile([128, NFT, THALF], BF16, tag="h", name="h_sb")
                # z2 halves are SEPARATE tiles so the A-half matmuls carry
                # no (false) dependency on the RS-B landing DMA
                z2A = fh.tile([128, NCH, TQ], BF16, tag="hzA", name="z2A")
                z2B = fh.tile([128, NCH, TQ], BF16, tag="hzB", name="z2B")
                nc.sync.dma_start(z2A[:], rs_outA.rearrange("c p t -> p c t"))
                # correction rows: row0 = -mean(z2), row1 = std(z2); the
                # stacked rank-2 matmul (corrw x rows) folds the layernorm
                # into the w_in matmul; rstd is deferred past the ReLU.
                rows = stat.tile([2, THALF], BF16, tag="rows", name="rows")
                rstd2_bc = fh.tile([128, THALF], F32, tag="r2bc",
                                   name="rstd2_bc")
                nmr2_bc = fh.tile([128, THALF], F32, tag="n2bc",
                                  name="nmr2_bc")

                def z2_half_stats(z2X, off):
                    tw = TQ
                    s_ps, ss_ps = stats_mms(z2X, tw)
                    s16 = row16(s_ps, tw, "s2r")
                    ss16 = row16(ss_ps, tw, "ss2r")
                    mean_r = stat.tile([1, THALF], F32, tag="mean_r", bufs=2,
                                       name="mean_r")
                    nc.vector.tensor_scalar(out=mean_r[:, :tw], in0=s16[:, :tw],
                                            scalar1=INV_D, scalar2=None,
                                            op0=ALU.mult)
                    var_r = stat.tile([1, THALF], F32, tag="var_r", bufs=2,
                                      name="var_r")
                    nc.vector.tensor_scalar(out=var_r[:, :tw],
                                            in0=ss16[:, :tw],
                                            scalar1=INV_D1, scalar2=None,
                                            op0=ALU.mult)
                    m2_r = stat.tile([1, THALF], F32, tag="m2_r", bufs=2,
                                     name="m2_r")
                    nc.vector.tensor_scalar(out=m2_r[:, :tw],
                                            in0=mean_r[:, :tw],
                                            scalar1=D_OVER_D1, scalar2=None,
                                            op0=ALU.mult)
                    nc.vector.tensor_tensor(m2_r[:, :tw], m2_r[:, :tw],
                                            mean_r[:, :tw], ALU.mult)
                    nc.vector.tensor_tensor(var_r[:, :tw], var_r[:, :tw],
                                            m2_r[:, :tw], ALU.subtract)
                    std_r = stat.tile([1, THALF], F32, tag="std_r", bufs=2,
                                      name="std_r")
                    nc.scalar.activation(out=std_r[:, :tw], in_=var_r[:, :tw],
                                         func=AF.Sqrt)
                    nc.vector.tensor_scalar(out=rows[0:1, off:off + tw],
                                            in0=mean_r[:, :tw], scalar1=-1.0,
                                            scalar2=None, op0=ALU.mult)
                    # DVE cannot write at partition base 1; stage + DMA
                    stdb = stat.tile([1, THALF], BF16, tag="stdb", bufs=2,
                                     name="stdb")
                    nc.vector.tensor_copy(out=stdb[:, :tw], in_=std_r[:, :tw])
                    nc.sync.dma_start(rows[1:2, off:off + tw], stdb[:, :tw])
                    rstd_f = stat.tile([1, THALF], F32, tag="rstd_f", bufs=2,
                                       name="rstd_f")
                    nc.vector.reciprocal(out=rstd_f[:, :tw], in_=std_r[:, :tw])
                    rstd_r = stat.tile([1, THALF], F16, tag="rstd_r", bufs=2,
                                       name="rstd_r")
                    nc.vector.tensor_copy(out=rstd_r[:, :tw],
                                          in_=rstd_f[:, :tw])
                    nmr_r = stat.tile([1, THALF], F16, tag="nmr_r", bufs=2,
                                      name="nmr_r")
                    nc.vector.tensor_mul(out=nmr_r[:, :tw],
                                         in0=rows[0:1, off:off + tw],
                                         in1=rstd_r[:, :tw])
                    rb_ps = ps_tile((128, THALF))
                    nc.tensor.matmul(rb_ps[:, :tw], ones_row[:],
                                     rstd_r[:, :tw], start=True, stop=True)
                    nc.vector.tensor_copy(out=rstd2_bc[:, off:off + tw],
                                          in_=rb_ps[:, :tw])
                    nb_ps = ps_tile((128, THALF))
                    nc.tensor.matmul(nb_ps[:, :tw], ones_row[:],
                                     nmr_r[:, :tw], start=True, stop=True)
                    nc.vector.tensor_copy(out=nmr2_bc[:, off:off + tw],
                                          in_=nb_ps[:, :tw])

                z2_half_stats(z2A, 0)
                nc.sync.dma_start(z2B[:], rs_outB.rearrange("c p t -> p c t"))

                pre_wout = fh.tile([128, NFT, 128], BF16, tag="wout", bufs=2,
                                   name="wout_t")
                nc.sync.dma_start(pre_wout[:],
                                  io["wout"].ap()[0].rearrange("f p m -> p f m"))

                win_tiles = {}

                def load_win(ft):
                    wt = fh.tile([128, NCH, 128], BF16, tag="win",
                                 bufs=NWIN, name="win_t")
                    nc.sync.dma_start(
                        wt[:], io["win"].ap()[ft].rearrange("c p f -> p c f"))
                    win_tiles[ft] = wt

                for ft in range(6):
                    load_win(ft)

                def ffn_tile(ft, z2X, lo):
                    hp = ps_tile((128, TQ))
                    win_t = win_tiles[ft]
                    for c in range(NCH):
                        nc.tensor.matmul(hp[:], win_t[:, c, :],
                                         z2X[:, c, :],
                                         start=(c == 0), stop=False)
                    nc.tensor.matmul(hp[:],
                                     corrw_sb[:, 128 * ft:128 * (ft + 1)],
                                     rows[:, lo:lo + TQ], start=False,
                                     stop=True)
                    nc.scalar.activation(out=h_sb[:, ft, lo:lo + TQ],
                                         in_=hp[:],
                                         func=AF.Relu)

                # A-quarter tiles, with B lagging by FFN_LAG (hides RS-B)
                for ft in range(NFT):
                    if ft + 6 < NFT:
                        load_win(ft + 6)
                    ffn_tile(ft, z2A, 0)
                    if ft == FFN_LAG - 2:
                        z2_half_stats(z2B, TQ)
                    if ft >= FFN_LAG:
                        ffn_tile(ft - FFN_LAG, z2B, TQ)
                for ft in range(NFT - FFN_LAG, NFT):
                    ffn_tile(ft, z2B, TQ)

                # wout + deferred-rstd z3 assembly; norm3 stats interleaved
                z3h = (z2A, z2B)  # in-place halves after assembly reads
                z3f = fh.tile([128, NCH, THALF], F32, tag="z3f", name="z3f")
                s3_ps = ps_tile((1, T))
                ss3_ps = ps_tile((1, T))
                zsq3 = {}

                def z3_stats(e, stop):
                    for i, zx in enumerate(z3h):
                        nc.tensor.matmul(s3_ps[:, TQ * i:TQ * (i + 1)],
                                         ones_col[:], zx[:, e, :],
                                         start=(e == 0), stop=stop)
                    zq = zsq3.pop(e)
                    nc.tensor.matmul(ss3_ps[:, :THALF], ones_col[:],
                                     zq[:, :THALF], start=(e == 0), stop=stop)

                for e in range(NCH):
                    if e == 0:
                        wout_t = pre_wout
                    else:
                        wout_t = fh.tile([128, NFT, 128], BF16,
                                         tag="wout", bufs=2, name="wout_t")
                        nc.sync.dma_start(
                            wout_t[:],
                            io["wout"].ap()[e].rearrange("f p m -> p f m"))
                    fp = ps_tile((128, THALF))
                    for fc in range(NFT):
                        nc.tensor.matmul(
                            fp[:], wout_t[:, fc, :], h_sb[:, fc, :],
                            start=(fc == 0), stop=(fc == NFT - 1))
                    # z3 = rstd2*(wout@relu + z2) + nmr2 + bout
                    t1 = scr.tile([128, T], F32, tag="s4", bufs=2,
                                  name="fftmp")
                    nc.vector.tensor_tensor(t1[:, 0:TQ], fp[:, 0:TQ],
                                            z2A[:, e, :], ALU.add)
                    nc.vector.tensor_tensor(t1[:, TQ:THALF], fp[:, TQ:THALF],
                                            z2B[:, e, :], ALU.add)
                    nc.vector.tensor_mul(out=z3f[:, e, :], in0=t1[:, :THALF],
                                         in1=rstd2_bc[:])
                    nc.vector.tensor_tensor(z3f[:, e, :], z3f[:, e, :],
                                            nmr2_bc[:], ALU.add)
                    nc.vector.tensor_scalar(out=z3f[:, e, :],
                                            in0=z3f[:, e, :],
                                            scalar1=bout_sb[:, e:e + 1],
                                            scalar2=None, op0=ALU.add)
                    nc.scalar.mul(z2A[:, e, :], z3f[:, e, 0:TQ], 1.0)
                    nc.scalar.mul(z2B[:, e, :], z3f[:, e, TQ:THALF], 1.0)
                    zq = scr.tile([128, T], BF16, tag="zsq", bufs=2,
                                  name="zsq3")
                    nc.vector.tensor_mul(out=zq[:, :THALF], in0=z3f[:, e, :],
                                         in1=z3f[:, e, :])
                    zsq3[e] = zq
                    if e >= 1:
                        z3_stats(e - 1, stop=False)
                z3_stats(NCH - 1, stop=True)
                _, rstd3_bc, nmr3_bc = norm_finish(s3_ps, ss3_ps, THALF)

                def write_out(c, oc):
                    nc.sync.dma_start(out_d.ap()[c][:, 0:TQ], oc[:, 0:TQ])
                    nc.sync.dma_start(out_d.ap()[c][:, TQ:THALF],
                                      oc[:, TQ:THALF])

                norm_apply(z3f, THALF, rstd3_bc, nmr3_bc, None,
                           chunk_writer=write_out, apply_src=z3f)


# ============================================================== host side ===
def _to_bf16(a):
    import ml_dtypes
    return np.asarray(a, np.float32).astype(ml_dtypes.bfloat16)


def _to_f8(a, scale):
    import ml_dtypes
    a = np.clip(np.asarray(a, np.float32) * scale, -240.0, 240.0)
    return a.astype(ml_dtypes.float8_e4m3)


def _prep_inputs(inputs):
    """Per-core in_maps (host does transposes/tiling/dtype casts only)."""
    x = np.asarray(inputs["x"], np.float32)
    y = np.asarray(inputs["y"], np.float32)
    mask = _to_bf16(np.tile(np.tril(np.ones((128, DK), np.float32)), (1, 2)))
    win_t = _to_bf16(np.asarray(inputs["w_in"], np.float32).T
                     .reshape(NCH, 128, NFT, 128).transpose(2, 0, 1, 3))
    wout_t = _to_bf16(np.asarray(inputs["w_out"], np.float32).T
                      .reshape(NFT, 128, NCH, 128).transpose(2, 0, 1, 3))
    corrw = _to_bf16(np.stack([
        np.asarray(inputs["w_in"], np.float32).sum(axis=1),
        np.asarray(inputs["b_in"], np.float32)]))
    bout2 = np.ascontiguousarray(
        np.asarray(inputs["b_out"], np.float32).reshape(NCH, 128).T)

    def packw(w, hs, f8=False):
        w = np.asarray(w, np.float32)[hs].transpose(1, 0, 2) \
            .reshape(D, 512).reshape(NCH, 128, 512)
        return _to_f8(w, SW) if f8 else _to_bf16(w)

    shared = {"win": win_t, "wout": wout_t, "corrw": corrw, "bout": bout2,
              "mask": mask}
    in_maps = []
    for c in range(NCORES):
        b, m = c // 2, c % 2
        hs = slice(8 * m, 8 * (m + 1))
        im = dict(shared)
        im["yT"] = _to_bf16(y[b].T.reshape(NCH, 128, T))
        im["y8T"] = _to_f8(y[b].T.reshape(NCH, 128, T), SA)
        im["x8T"] = _to_f8(x[b].T.reshape(NCH, 128, T), SA)
        im["wq1"] = packw(inputs["Wq1"], hs)
        im["wk1"] = packw(inputs["Wk1"], hs)
        im["wv1"] = packw(inputs["Wv1"], hs, f8=True)
        im["wq2"] = packw(inputs["Wq2"], hs, f8=True)
        im["wk2"] = packw(inputs["Wk2"], hs, f8=True)
        im["wv2"] = packw(inputs["Wv2"], hs, f8=True)
        im["wo1"] = _to_f8(np.asarray(inputs["Wo1"], np.float32)
                           .reshape(2 * NPAIR, 128, D), SW)
        im["wo2"] = _to_f8(np.asarray(inputs["Wo2"], np.float32)
                           [512 * m:512 * (m + 1)].reshape(NPAIR, 128, D), SW)
        in_maps.append(im)
    return in_maps


def _assemble(results):
    out3 = np.empty((B, T, D), np.float32)
    for b in range(B):
        halves = [results[2 * b + m]["out3T"].reshape(D, THALF)
                  for m in range(2)]
        out3[b] = np.concatenate(halves, axis=1).T
    return out3


# ================================================================ runner ===
_CACHE = {}


def _make_runner(nc, n_cores):
    import jax
    from jax.sharding import Mesh, PartitionSpec
    from jax.experimental.shard_map import shard_map
    from concourse.bass2jax import (_bass_exec_p, install_neuronx_cc_hook,
                                    partition_id_tensor)

    install_neuronx_cc_hook()
    partition_name = nc.partition_id_tensor.name if nc.partition_id_tensor else None
    in_names, out_names, out_avals, zero_outs = [], [], [], []
    for alloc in nc.m.functions[0].allocations:
        if not isinstance(alloc, mybir.MemoryLocationSet):
            continue
        name = alloc.memorylocations[0].name
        if alloc.kind == "ExternalInput":
            if name != partition_name:
                in_names.append(name)
        elif alloc.kind == "ExternalOutput":
            shape = tuple(alloc.tensor_shape)
            dtype = mybir.dt.np(alloc.dtype)
            out_names.append(name)
            out_avals.append(jax.core.ShapedArray(shape, dtype))
            zero_outs.append(np.zeros(shape, dtype))
    n_params = len(in_names)
    n_outs = len(out_avals)
    all_in = in_names + out_names + ([partition_name] if partition_name else [])

    def _body(*args):
        operands = list(args)
        if partition_name is not None:
            operands.append(partition_id_tensor())
        return tuple(_bass_exec_p.bind(
            *operands, out_avals=tuple(out_avals), in_names=tuple(all_in),
            out_names=tuple(out_names), lowering_input_output_aliases=(),
            sim_require_finite=True, sim_require_nnan=True, nc=nc))

    devices = jax.devices()[:n_cores]
    mesh = Mesh(np.asarray(devices), ("core",))
    sharded = jax.jit(
        shard_map(_body, mesh=mesh,
                  in_specs=(PartitionSpec("core"),) * (n_params + n_outs),
                  out_specs=(PartitionSpec("core"),) * n_outs,
                  check_rep=False),
        keep_unused=True)

    def run(in_maps):
        concat_in = [
            np.concatenate([np.asarray(in_maps[c][nm]) for c in range(n_cores)],
                           axis=0)
            for nm in in_names
        ]
        concat_zero = [np.concatenate([z] * n_cores, axis=0) for z in zero_outs]
        outs = [np.asarray(o) for o in sharded(*concat_in, *concat_zero)]
        results = []
        for c in range(n_cores):
            r = {}
            for i, nm in enumerate(out_names):
                per = outs[i].shape[0] // n_cores
                r[nm] = outs[i][c * per:(c + 1) * per]
            results.append(r)
        return results

    return run


def _get_built(debug=False):
    key = "dbg" if debug else "main"
    if key not in _CACHE:
        nc = build_nc(debug=debug)
        run = _make_runner(nc, NCORES)
        _CACHE[key] = (nc, run)
    return _CACHE[key]


def kernel(**inputs):
    nc, run = _get_built()
    in_maps = _prep_inputs(inputs)
    results = run(in_maps)
    out3 = _assemble(results)
    return (np.asarray(inputs["x"], np.float32), out3)
